# revision 1
# baseline (speedup 1.0000x reference)
"""BiLSTM-CRF loss for nn_BiLSTM_CRF_68152541053203 on 8 TRN2 NeuronCores.

Sharding: batch x direction. B=64 splits into 4 groups of 16 sequences; each
group gets a core pair: core 2g runs the forward word-LSTM direction, core
2g+1 the backward direction (on host-time-flipped inputs, so the SPMD kernel
is identical). Each core computes on-device, SBUF-resident:
    xg = x @ Wih_d.T + b_d          (gate-chunk-major, fp8 in, f32 psum)
    single-direction LSTM recurrence, 256 steps, fp8 DoubleRow matmuls
    em_part = seq_d @ Wtag_d.T      -> (20, 4096) fp8
Host: char BiLSTM + embedding gather (tiny), sums the two partial emissions
per group (+btag, bwd part time-unflipped), and runs the CRF forward scan
(generic mask support).

Device layouts (per core, BL2=16 seqs, T=256, NT=4096):
  xT    (321, NT) fp8e4m3: col = b*T + t; rows 0:320 features, row 320 ones
        (bias row trick); bwd cores receive x time-reversed
  wih   (321, 1024) fp8e4m3: cols = this direction's gates, order [i,f,o,g]
  whhT  (128, 2048) fp8e4m3: [p, k*1024+g] = WhhT_d[k*128+p, g] (DoubleRow)
  wtagT (256, 20)  fp8e4m3: this direction's 256 rows of Wtag.T
  em    (20, NT)   fp8e4m3 output (partial emissions, no btag)
On-chip: xgT (128, 8*NT) bf16; seq (128, 2*NT) fp8 (col = k*NT + b*T + t),
which doubles as the next step's matmul rhs; gate/cell state (128, 160) f32.
The per-step Whh matmuls write one PSUM mega-tile (one accumulation group
per bank), read back as a single strided AP by one vector add.
"""

import numpy as np

import concourse.bacc as bacc
import concourse.mybir as mybir
import concourse.tile as tile

N_CORES = 8
B, T = 64, 256
CIN, CH = 25, 10
EMB_IN, H = 320, 256
K = 20
BL2 = 16                      # sequences per core (4 groups x 2 dirs)
NT = BL2 * T                  # 4096
KD = EMB_IN + 1
AF = mybir.ActivationFunctionType

_CACHE = {}


def _build_nc():
    bf = mybir.dt.bfloat16
    f8 = mybir.dt.float8e4
    f32 = mybir.dt.float32
    NB = NT // 512
    nc = bacc.Bacc("TRN2", target_bir_lowering=False, debug=False,
                   num_devices=N_CORES)
    xT = nc.dram_tensor("xT", [KD, NT], f8, kind="ExternalInput").ap()
    wih = nc.dram_tensor("wih", [KD, 1024], f8, kind="ExternalInput").ap()
    whhT = nc.dram_tensor("whhT", [128, 2048], f8, kind="ExternalInput").ap()
    wtagT = nc.dram_tensor("wtagT", [H, K], f8, kind="ExternalInput").ap()
    em = nc.dram_tensor("em", [K, NT], f8, kind="ExternalOutput").ap()

    KT = [(0, 128), (128, 128), (256, 65)]
    with tile.TileContext(nc) as tc:
        with (
            tc.tile_pool(name="per", bufs=1) as per,
            tc.tile_pool(name="st", bufs=2) as st,
        ):
            wih01 = per.tile([128, 2 * 1024], f8, tag="wih01", name="wih01")
            nc.gpsimd.dma_start(wih01[:, 0:1024], wih[0:128, :])
            nc.gpsimd.dma_start(wih01[:, 1024:2048], wih[128:256, :])
            wih01r = wih01[:].rearrange("p (k g) -> p k g", k=2)
            wih2 = per.tile([65, 1024], f8, tag="wih2", name="wih2")
            nc.gpsimd.dma_start(wih2[:], wih[256:321, :])
            xT01 = per.tile([128, 2 * NT], f8, tag="xT01", name="xT01")
            nc.gpsimd.dma_start(xT01[:, 0:NT], xT[0:128, :])
            nc.gpsimd.dma_start(xT01[:, NT:2 * NT], xT[128:256, :])
            xT01r = xT01[:].rearrange("p (k n) -> p k n", k=2)
            xT2 = per.tile([65, NT], f8, tag="xT2", name="xT2")
            nc.gpsimd.dma_start(xT2[:], xT[256:321, :])
            whh8 = per.tile([128, 2048], f8, tag="whh8", name="whh8")
            nc.gpsimd.dma_start(whh8[:], whhT[:, :])
            whh8r = whh8[:].rearrange("p (k g) -> p k g", k=2)
            wt8 = per.tile([128, 2 * K], f8, tag="wt8", name="wt8")
            nc.gpsimd.dma_start(wt8[:, 0:K], wtagT[0:128, :])
            nc.gpsimd.dma_start(wt8[:, K:2 * K], wtagT[128:256, :])
            wt8r = wt8[:].rearrange("p (k n) -> p k n", k=2)

            xgT = per.tile([128, 8 * NT], bf, tag="xgT", name="xgT")
            sq = per.tile([128, 2 * NT], f8, tag="sq", name="sq")
            sq4 = sq[:].rearrange("p (k b t) -> p k b t", k=2, b=BL2)
            # gac: i 0:32 | f 32:64 | o 64:96 | g 96:128 | c 128:160
            gac = per.tile([128, 160], f32, tag="gac", name="gac")
            ga = gac[:, 0:128]
            cT = gac[:, 128:160]

            # xg projection: xgT[128, gc*NT + b*T + t]
            with tc.tile_pool(name="pm", bufs=2, space="PSUM") as pm:
                for gc in range(8):
                    for nb in range(NB):
                        ps = pm.tile([128, 512], f32, tag="xps", name="xps")
                        nc.tensor.matmul(
                            ps[:],
                            wih01r[:, :, gc * 128:(gc + 1) * 128],
                            xT01r[:, :, nb * 512:(nb + 1) * 512],
                            perf_mode=mybir.MatmulPerfMode.DoubleRow,
                            start=True, stop=False,
                        )
                        nc.tensor.matmul(
                            ps[:],
                            wih2[:, gc * 128:(gc + 1) * 128],
                            xT2[:, nb * 512:(nb + 1) * 512],
                            start=False, stop=True,
                        )
                        dst = xgT[:, gc * NT + nb * 512:gc * NT + (nb + 1) * 512]
                        if (gc * NB + nb) % 2 == 0:
                            nc.vector.tensor_copy(dst, ps[:])
                        else:
                            nc.scalar.copy(dst, ps[:])

            # LSTM recurrence; gate cols of ga: i 0:32, f 32:64, o 64:96,
            # g 96:128 (16 batch cols per half-chunk)
            xg4 = xgT[:].rearrange("p (g b t) -> p g b t", g=8, b=BL2)
            with tc.tile_pool(name="prm", bufs=1, space="PSUM") as prm:
                # one PSUM tile spanning all 8 banks: chunk c's accumulation
                # group lives in bank c, readable back as one strided AP
                psall = prm.tile([128, 4096], f32, tag="psall", name="psall")
                psr = psall[:].rearrange("p (c n) -> p c n", c=8)[:, :, 0:16]
                for t in range(T):
                    if t == 0:
                        nc.scalar.activation(ga[:, 0:96],
                                             xg4[:, 0:6, :, t], AF.Sigmoid)
                        nc.scalar.activation(ga[:, 96:128],
                                             xg4[:, 6:8, :, t], AF.Tanh)
                        nc.vector.tensor_mul(cT, ga[:, 0:32], ga[:, 96:128])
                    else:
                        gs = st.tile([128, 128], f32, tag="gs", name="gs")
                        for c in range(8):
                            nc.tensor.matmul(
                                psall[:, c * 512:c * 512 + 16],
                                whh8r[:, :, c * 128:(c + 1) * 128],
                                sq4[:, :, :, t - 1],
                                perf_mode=mybir.MatmulPerfMode.DoubleRow,
                            )
                        nc.vector.tensor_add(gs[:], psr, xg4[:, :, :, t])
                        nc.scalar.activation(ga[:, 0:96], gs[:, 0:96],
                                             AF.Sigmoid)
                        nc.scalar.activation(ga[:, 96:128], gs[:, 96:128],
                                             AF.Tanh)
                        # [i*g | f*c] in one mul: (i,f) x (g,c) contiguous
                        t12 = st.tile([128, 64], f32, tag="t12", name="t12")
                        nc.vector.tensor_mul(t12[:], gac[:, 0:64],
                                             gac[:, 96:160])
                        nc.vector.tensor_add(cT, t12[:, 0:32], t12[:, 32:64])
                    th = st.tile([128, 32], f32, tag="th", name="th")
                    nc.scalar.activation(th[:], cT, AF.Tanh)
                    nc.vector.tensor_mul(sq4[:, :, :, t], ga[:, 64:96], th[:])

            # partial emissions: em = seq_d @ Wtag_d.T (DoubleRow over k)
            em_sb = per.tile([K, NT], f8, tag="em", name="em")
            with tc.tile_pool(name="pe", bufs=2, space="PSUM") as pe:
                for nb in range(NB):
                    ps = pe.tile([K, 512], f32, tag="eps", name="eps")
                    for k in range(2):
                        nc.tensor.matmul(
                            ps[:], wt8r[:, k, :],
                            sq[:, k * NT + nb * 512:k * NT + (nb + 1) * 512],
                            start=(k == 0), stop=(k == 1),
                        )
                    if nb % 2 == 0:
                        nc.vector.tensor_copy(em_sb[:, nb * 512:(nb + 1) * 512],
                                              ps[:])
                    else:
                        nc.scalar.copy(em_sb[:, nb * 512:(nb + 1) * 512], ps[:])
                nc.gpsimd.dma_start(em[:, :], em_sb[:])
    nc.compile()
    return nc


def _make_runner(nc):
    """Build the jitted PJRT executor once (run_bass_via_pjrt re-traces on
    every call; caching the jit + shard_map saves that per-call cost)."""
    import jax
    from jax.sharding import Mesh, PartitionSpec
    from jax.experimental.shard_map import shard_map
    from concourse import bass2jax
    bass2jax.install_neuronx_cc_hook()
    assert nc.dbg_addr is None
    pid_name = (nc.partition_id_tensor.name
                if nc.partition_id_tensor else None)

    in_names, out_names, out_avals, zero_outs = [], [], [], []
    for alloc in nc.m.functions[0].allocations:
        if not isinstance(alloc, mybir.MemoryLocationSet):
            continue
        name = alloc.memorylocations[0].name
        if alloc.kind == "ExternalInput":
            if name != pid_name:
                in_names.append(name)
        elif alloc.kind == "ExternalOutput":
            out_names.append(name)
            shape = tuple(alloc.tensor_shape)
            dtype = mybir.dt.np(alloc.dtype)
            out_avals.append(jax.core.ShapedArray(shape, dtype))
            zero_outs.append(np.zeros(shape, dtype))
    n_params = len(in_names)
    donate = tuple(range(n_params, n_params + len(out_names)))
    all_names = in_names + out_names
    if pid_name is not None:
        all_names = all_names + [pid_name]

    def _body(*args):
        operands = list(args)
        if pid_name is not None:
            operands.append(bass2jax.partition_id_tensor())
        outs = bass2jax._bass_exec_p.bind(
            *operands, out_avals=tuple(out_avals), in_names=tuple(all_names),
            out_names=tuple(out_names), lowering_input_output_aliases=(),
            sim_require_finite=True, sim_require_nnan=True, nc=nc)
        return tuple(outs)

    devices = jax.devices()[:N_CORES]
    mesh = Mesh(np.asarray(devices), ("core",))
    specs = (PartitionSpec("core"),) * (n_params + len(out_names))
    sharded = jax.jit(
        shard_map(_body, mesh=mesh, in_specs=specs,
                  out_specs=(PartitionSpec("core"),) * len(out_names),
                  check_rep=False),
        donate_argnums=donate, keep_unused=True)

    import jax.numpy as jnp
    out_sharding = jax.sharding.NamedSharding(mesh, PartitionSpec("core"))

    def run(in_maps):
        if isinstance(in_maps, dict):        # pre-assembled sharded slabs
            concat_in = [in_maps[n] for n in in_names]
        else:
            concat_in = [np.concatenate([m[n] for m in in_maps], axis=0)
                         for n in in_names]
        concat_zeros = [jnp.zeros((N_CORES * z.shape[0], *z.shape[1:]),
                                  z.dtype, device=out_sharding)
                        for z in zero_outs]
        outs = sharded(*concat_in, *concat_zeros)
        return [{n: np.asarray(outs[i]).reshape(
                    N_CORES, *out_avals[i].shape)[c]
                 for i, n in enumerate(out_names)}
                for c in range(N_CORES)]

    return run


def _run_device(in_maps):
    if "runner" not in _CACHE:
        _CACHE["runner"] = _make_runner(_CACHE["nc"])
    return _CACHE["runner"](in_maps)


def _perm_gates(W):
    """PyTorch gate-row order [i,f,g,o] -> kernel order [i,f,o,g].

    W: (1024, ...) single-direction gate-stacked array."""
    return np.concatenate([W[0:512], W[768:1024], W[512:768]], axis=0)


def _sigmoid(x):
    return 1.0 / (1.0 + np.exp(-x))


def _lstm_dir_host(x, Wih, Whh, b):
    """Small (char) LSTM on host. x: (B,T,I) -> (B,T,Hd)."""
    xg = np.einsum('bti,gi->btg', x, Wih, optimize=True) + b
    xg = xg.astype(np.float32)
    Bs, Ts, G = xg.shape
    Hd = G // 4
    WhhT = np.ascontiguousarray(Whh.T)
    h = np.zeros((Bs, Hd), np.float32)
    c = np.zeros((Bs, Hd), np.float32)
    out = np.empty((Bs, Ts, Hd), np.float32)
    for t in range(Ts):
        g = xg[:, t] + h @ WhhT
        i = _sigmoid(g[:, :Hd])
        f = _sigmoid(g[:, Hd:2 * Hd])
        gg = np.tanh(g[:, 2 * Hd:3 * Hd])
        o = _sigmoid(g[:, 3 * Hd:])
        c = f * c + i * gg
        h = o * np.tanh(c)
        out[:, t] = h
    return out


def _logsumexp(a, axis):
    m = np.max(a, axis=axis, keepdims=True)
    return (m + np.log(np.sum(np.exp(a - m), axis=axis,
                              keepdims=True))).squeeze(axis)


def _pack_dir(x_grp, Wih_d, b_d, Whh_d, Wtag_rows, flip):
    """Build one core's in_map. x_grp: (16,T,320) f32."""
    import ml_dtypes
    f8np = ml_dtypes.float8_e4m3
    xs = x_grp[:, ::-1] if flip else x_grp
    xTm = np.empty((KD, NT), np.float32)
    xTm[:320] = np.ascontiguousarray(xs).reshape(NT, EMB_IN).T
    xTm[320] = 1.0
    wihm = np.empty((KD, 1024), np.float32)
    wihm[:320] = _perm_gates(Wih_d).T
    wihm[320] = _perm_gates(b_d.reshape(-1, 1))[:, 0]
    whhm = _perm_gates(Whh_d).T                      # (256, 1024)
    whh8 = np.ascontiguousarray(
        whhm.reshape(2, 128, 1024).transpose(1, 0, 2).reshape(128, 2048)
    ).astype(f8np)
    return {"xT": xTm.astype(f8np), "wih": wihm.astype(f8np), "whhT": whh8,
            "wtagT": np.ascontiguousarray(Wtag_rows).astype(f8np)}


def kernel(char_tensor, token_tensor, tags, mask, emb,
           cWih_f, cWhh_f, cb_f, cWih_b, cWhh_b, cb_b,
           wWih_f, wWhh_f, wb_f, wWih_b, wWhh_b, wb_b,
           Wtag, btag, start_t, end_t, trans):
    f32 = lambda a: np.asarray(a, np.float32)
    char_tensor = f32(char_tensor)
    emb = f32(emb)
    token_tensor = np.asarray(token_tensor).astype(np.int64)
    tags_i = np.asarray(tags).astype(np.int64)
    mask_b = np.asarray(mask).astype(bool)

    # --- char BiLSTM (tiny) + embedding gather on host ---
    cf = _lstm_dir_host(char_tensor, f32(cWih_f), f32(cWhh_f), f32(cb_f))
    cb = _lstm_dir_host(char_tensor[:, ::-1], f32(cWih_b), f32(cWhh_b),
                        f32(cb_b))[:, ::-1]
    word_emb = emb[token_tensor]                                  # (B,T,300)
    x = np.concatenate([cf, cb, word_emb], axis=2)                # (B,T,320)

    WtagT = np.ascontiguousarray(f32(Wtag).T)                     # (512, 20)
    per_core = []
    for g in range(4):
        xg_ = x[g * BL2:(g + 1) * BL2]
        per_core.append(_pack_dir(xg_, f32(wWih_f), f32(wb_f), f32(wWhh_f),
                                  WtagT[0:256], flip=False))
        per_core.append(_pack_dir(xg_, f32(wWih_b), f32(wb_b), f32(wWhh_b),
                                  WtagT[256:512], flip=True))
    # assemble the sharded slabs once here (host prep) so the device call
    # does no per-call concatenation
    in_maps = {n: np.concatenate([m[n] for m in per_core], axis=0)
               for n in per_core[0]}

    if "nc" not in _CACHE:
        _CACHE["nc"] = _build_nc()
    _CACHE["last_in_maps"] = in_maps
    # First exec on a freshly-compiled NEFF occasionally hits a transient
    # failure on this axon tunnel; retry (fresh build on second failure).
    res = None
    for attempt in range(3):
        try:
            res = _run_device(in_maps)
            break
        except Exception:
            if attempt == 2:
                raise
            import time as _time
            _time.sleep(5)
            _CACHE.pop("runner", None)
            if attempt == 1:
                _CACHE.pop("nc", None)
                _CACHE["nc"] = _build_nc()

    em = np.empty((B, T, K), np.float32)
    for g in range(4):
        ef = np.asarray(res[2 * g]["em"], np.float32)
        eb = np.asarray(res[2 * g + 1]["em"], np.float32)
        ef = ef.T.reshape(BL2, T, K)
        eb = eb.T.reshape(BL2, T, K)[:, ::-1]       # un-flip time
        em[g * BL2:(g + 1) * BL2] = ef + eb
    em += f32(btag)

    # --- CRF NLL on host (generic mask support) ---
    em = np.swapaxes(em, 0, 1)                                    # (T,B,K)
    tg = np.swapaxes(tags_i, 0, 1)
    m = np.swapaxes(mask_b, 0, 1).astype(np.float32)
    start_t, end_t, trans = f32(start_t), f32(end_t), f32(trans)
    bidx = np.arange(B)
    e_sc = np.take_along_axis(em, tg[..., None], axis=-1)[..., 0]  # (T,B)
    num = start_t[tg[0]] + e_sc[0]
    num = num + np.sum((trans[tg[:-1], tg[1:]] + e_sc[1:]) * m[1:], axis=0)
    last = (np.sum(m, axis=0) - 1).astype(np.int64)
    num = num + end_t[tg[last, bidx]]
    alpha = start_t[None, :] + em[0]
    for t in range(1, T):
        nxt = _logsumexp(alpha[:, :, None] + trans[None, :, :]
                         + em[t][:, None, :], axis=1)
        alpha = np.where(m[t][:, None] > 0, nxt, alpha)
    den = _logsumexp(alpha + end_t[None, :], axis=1)
    return np.float32(-np.sum(num - den))



# revision 6
# speedup vs baseline: 341.2062x; 341.2062x over previous
"""BiLSTM-CRF loss for nn_BiLSTM_CRF_68152541053203 on 8 TRN2 NeuronCores.

Sharding: batch x direction. B=64 splits into 4 groups of 16 sequences; each
group gets a core pair: core 2g runs the forward word-LSTM direction, core
2g+1 the backward direction (on host-time-flipped inputs, so the SPMD kernel
is identical). Each core computes on-device, SBUF-resident:
    xg = x @ Wih_d.T + b_d          (projected in 8-step chunks into PSUM)
    single-direction LSTM recurrence, 256 steps, fp8 DoubleRow matmuls
    em_part = seq_d @ Wtag_d.T      -> (20, 4096) fp8
Host: char BiLSTM + embedding gather (tiny), sums the two partial emissions
per group (+btag, bwd part time-unflipped), and runs the CRF forward scan
(generic mask support).

Key performance structure (v2):
  * Two independent 8-seq chains per core (A = seqs 0-7, B = 8-15),
    interleaved instruction-by-instruction so one chain's vector work hides
    the other's scalar-engine latency and cross-engine sync gaps.
  * The input projection xg = x@Wih.T runs on the (otherwise idle) PE in
    8-step chunks, directly into PSUM (start=True); the per-step Whh
    recurrence matmul then accumulates on top (start=False - PSUM has
    per-element has_written bits), so no separate gate-sum add is needed
    and the Activation engine reads gate pre-activations straight from
    PSUM. Chunks are double-buffered (4 PSUM banks total: chain x parity).
  * tanh(x) = 2*sigmoid(2x) - 1: the factor 2 is folded into the g-gate
    rows of Wih/Whh/bias on the host, so ONE sigmoid activation covers all
    four gates; a fused DVE tensor_scalar (2*s - 1) recovers tanh.

Device layouts (per core, chain width CW=8 seqs, T=256, chain cols NC2=2048):
  xT    (321, 4096) fp8e4m3: col = chain*2048 + t*8 + b (time-major within
        chain); rows 0:320 features, row 320 ones (bias row trick); bwd
        cores receive x time-reversed
  wih   (321, 1024) fp8e4m3: cols = this direction's gates, order [i,f,o,g],
        g-gate cols pre-scaled by 2
  whhT  (128, 2048) fp8e4m3: [p, k*1024+g] = WhhT_d[k*128+p, g] (DoubleRow),
        g-gate cols pre-scaled by 2
  wtagT (256, 20)  fp8e4m3: this direction's 256 rows of Wtag.T
  em    (20, 4096) fp8e4m3 output (partial emissions, no btag)
On-chip: sq (128, 2*4096) fp8 (col = k*4096 + chain*2048 + t*8 + b), which
doubles as the next step's matmul rhs and the emission matmul rhs;
gate/cell state f32 tiles per chain.
"""

import numpy as np

import concourse.bacc as bacc
import concourse.mybir as mybir
import concourse.tile as tile

N_CORES = 8
B, T = 64, 256
CIN, CH = 25, 10
EMB_IN, H = 320, 256
K = 20
BL2 = 16                      # sequences per core (4 groups x 2 dirs)
CW = 8                        # sequences per chain (2 chains per core)
NC2 = CW * T                  # 2048 cols per chain
NT = BL2 * T                  # 4096
KD = EMB_IN + 1
S = 8                         # steps per PSUM projection chunk
NG = T // S                   # 32 chunks
AF = mybir.ActivationFunctionType

_CACHE = {}


def _build_nc(repeat=1):
    from concourse.alu_op_type import AluOpType as ALU
    bf = mybir.dt.bfloat16
    f8 = mybir.dt.float8e4
    f32 = mybir.dt.float32
    nc = bacc.Bacc("TRN2", target_bir_lowering=False, debug=False,
                   num_devices=N_CORES)
    xT = nc.dram_tensor("xT", [KD, NT], f8, kind="ExternalInput").ap()
    wih = nc.dram_tensor("wih", [KD, 1024], f8, kind="ExternalInput").ap()
    whhT = nc.dram_tensor("whhT", [128, 2048], f8, kind="ExternalInput").ap()
    wtagT = nc.dram_tensor("wtagT", [H, K], f8, kind="ExternalInput").ap()
    em = nc.dram_tensor("em", [K, NT], f8, kind="ExternalOutput").ap()

    with tile.TileContext(nc) as tc:
        for _rep in range(repeat):
            _emit_body(nc, tc, xT, wih, whhT, wtagT, em, f8, f32, ALU)
    nc.compile()
    return nc


def _emit_body(nc, tc, xT, wih, whhT, wtagT, em, f8, f32, ALU):
    DR = mybir.MatmulPerfMode.DoubleRow
    with (
        tc.tile_pool(name="per", bufs=1) as per,
        tc.tile_pool(name="st", bufs=2) as st,
    ):
        wih01 = per.tile([128, 2 * 1024], f8, tag="wih01", name="wih01")
        nc.gpsimd.dma_start(wih01[:, 0:1024], wih[0:128, :])
        nc.gpsimd.dma_start(wih01[:, 1024:2048], wih[128:256, :])
        wih01r = wih01[:].rearrange("p (k g) -> p k g", k=2)
        wih2 = per.tile([65, 1024], f8, tag="wih2", name="wih2")
        nc.gpsimd.dma_start(wih2[:], wih[256:321, :])
        xT01 = per.tile([128, 2 * NT], f8, tag="xT01", name="xT01")
        nc.gpsimd.dma_start(xT01[:, 0:NT], xT[0:128, :])
        nc.gpsimd.dma_start(xT01[:, NT:2 * NT], xT[128:256, :])
        xT01r = xT01[:].rearrange("p (k n) -> p k n", k=2)
        xT2 = per.tile([65, NT], f8, tag="xT2", name="xT2")
        nc.gpsimd.dma_start(xT2[:], xT[256:321, :])
        whh8 = per.tile([128, 2048], f8, tag="whh8", name="whh8")
        nc.gpsimd.dma_start(whh8[:], whhT[:, :])
        whh8r = whh8[:].rearrange("p (k g) -> p k g", k=2)
        wt8 = per.tile([128, 2 * K], f8, tag="wt8", name="wt8")
        nc.gpsimd.dma_start(wt8[:, 0:K], wtagT[0:128, :])
        nc.gpsimd.dma_start(wt8[:, K:2 * K], wtagT[128:256, :])
        wt8r = wt8[:].rearrange("p (k n) -> p k n", k=2)

        # h history; col = k*4096 + chain*2048 + t*8 + b
        sq = per.tile([128, 2 * NT], f8, tag="sq", name="sq")
        sq4 = sq[:].rearrange("p (k c n) -> p k c n", k=2, c=2)
        # per-chain gate tiles: [i 0:16 | f 16:32 | o 32:48 | g 48:64]
        ga2 = per.tile([128, 128], f32, tag="ga2", name="ga2")
        # per-chain [ghat 0:16 | c 16:32]
        gc2 = per.tile([128, 64], f32, tag="gc2", name="gc2")
        th2 = per.tile([128, 32], f32, tag="th2", name="th2")
        nc.vector.memset(gc2[:, 16:32], 0.0)
        nc.vector.memset(gc2[:, 48:64], 0.0)

        with tc.tile_pool(name="prm", bufs=1, space="PSUM") as prm:
            # 4 separate one-bank tiles: q = chain*2 + (group parity); each
            # holds an 8-step chunk of gate pre-activations
            # [c*64 + t_rel*8 + b]. Separate tiles (not one 4-bank tile) so
            # tile-granular dependency tracking doesn't serialize one
            # chain's activation behind the other chain's / next chunk's
            # projection matmuls.
            psq = [prm.tile([128, 512], f32, tag=f"ps{q}", name=f"ps{q}")
                   for q in range(4)]
            psvq = [p[:] for p in psq]
            psrq = [p[:].rearrange("p (c n) -> p c n", c=8) for p in psq]

            def proj(chain, g, c):
                """Project xg for chunk g (8 steps), gate-chunk c, into
                PSUM bank chain*2 + g%2."""
                q = chain * 2 + (g % 2)
                dst = psvq[q][:, c * 64:(c + 1) * 64]
                cols = slice(chain * NC2 + g * 64, chain * NC2 + (g + 1) * 64)
                nc.tensor.matmul(dst, wih01r[:, :, c * 128:(c + 1) * 128],
                                 xT01r[:, :, cols], perf_mode=DR,
                                 start=True, stop=False)
                nc.tensor.matmul(dst, wih2[:, c * 128:(c + 1) * 128],
                                 xT2[:, cols], start=False, stop=True)

            for c in range(8):
                proj(0, 0, c)
                proj(1, 0, c)

            for g in range(NG):
                for tr in range(S):
                    t = g * S + tr
                    q0, q1 = 0 * 2 + (g % 2), 1 * 2 + (g % 2)
                    # --- PE: recurrence matmuls accumulate onto xg ---
                    if t > 0:
                        for chain, q in ((0, q0), (1, q1)):
                            for c in range(8):
                                nc.tensor.matmul(
                                    psvq[q][:, c * 64 + tr * 8:
                                            c * 64 + tr * 8 + 8],
                                    whh8r[:, :, c * 128:(c + 1) * 128],
                                    sq4[:, :, chain, (t - 1) * 8:t * 8],
                                    perf_mode=DR, start=False, stop=True,
                                    skip_group_check=True)
                    # --- PE: next chunk's projection, spread over steps ---
                    if g + 1 < NG:
                        proj(0, g + 1, tr)
                        proj(1, g + 1, tr)
                    # --- Act: one sigmoid over all gates (2x folded in g) ---
                    for chain, q in ((0, q0), (1, q1)):
                        nc.scalar.activation(
                            ga2[:, chain * 64:chain * 64 + 64],
                            psrq[q][:, :, tr * 8:(tr + 1) * 8], AF.Sigmoid)
                    # --- DVE chain A, then Act tanh A, then DVE chain B ---
                    for chain in (0, 1):
                        CB = chain * 64
                        GB = chain * 32
                        gh = gc2[:, GB:GB + 16]
                        cc = gc2[:, GB + 16:GB + 32]
                        nc.vector.tensor_scalar(
                            gh, ga2[:, CB + 48:CB + 64], 2.0, -1.0,
                            ALU.mult, ALU.add)
                        t12 = st.tile([128, 32], f32, tag=f"t12{chain}",
                                      name=f"t12{chain}")
                        nc.vector.tensor_mul(t12[:], ga2[:, CB:CB + 32],
                                             gc2[:, GB:GB + 32])
                        nc.vector.tensor_add(cc, t12[:, 0:16], t12[:, 16:32])
                        nc.scalar.activation(th2[:, GB // 2:GB // 2 + 16],
                                             cc, AF.Tanh)
                    for chain in (0, 1):
                        CB = chain * 64
                        nc.vector.tensor_mul(
                            sq4[:, :, chain, t * 8:(t + 1) * 8],
                            ga2[:, CB + 32:CB + 48],
                            th2[:, chain * 16:chain * 16 + 16])

        # partial emissions: em = seq_d @ Wtag_d.T (DoubleRow over k)
        em_sb = per.tile([K, NT], f8, tag="em", name="em")
        with tc.tile_pool(name="pe", bufs=2, space="PSUM") as pe:
            for chain in range(2):
                for nb in range(4):
                    ps = pe.tile([K, 512], f32, tag="eps", name="eps")
                    for k in range(2):
                        nc.tensor.matmul(
                            ps[:], wt8r[:, k, :],
                            sq4[:, k, chain, nb * 512:(nb + 1) * 512],
                            start=(k == 0), stop=(k == 1))
                    dst = em_sb[:, chain * NC2 + nb * 512:
                                chain * NC2 + (nb + 1) * 512]
                    if nb % 2 == 0:
                        nc.vector.tensor_copy(dst, ps[:])
                    else:
                        nc.scalar.copy(dst, ps[:])
            nc.gpsimd.dma_start(em[:, :], em_sb[:])


def _make_runner(nc):
    """Build the jitted PJRT executor once (run_bass_via_pjrt re-traces on
    every call; caching the jit + shard_map saves that per-call cost)."""
    import jax
    from jax.sharding import Mesh, PartitionSpec
    from jax.experimental.shard_map import shard_map
    from concourse import bass2jax
    bass2jax.install_neuronx_cc_hook()
    assert nc.dbg_addr is None
    pid_name = (nc.partition_id_tensor.name
                if nc.partition_id_tensor else None)

    in_names, out_names, out_avals, zero_outs = [], [], [], []
    for alloc in nc.m.functions[0].allocations:
        if not isinstance(alloc, mybir.MemoryLocationSet):
            continue
        name = alloc.memorylocations[0].name
        if alloc.kind == "ExternalInput":
            if name != pid_name:
                in_names.append(name)
        elif alloc.kind == "ExternalOutput":
            out_names.append(name)
            shape = tuple(alloc.tensor_shape)
            dtype = mybir.dt.np(alloc.dtype)
            out_avals.append(jax.core.ShapedArray(shape, dtype))
            zero_outs.append(np.zeros(shape, dtype))
    n_params = len(in_names)
    all_names = in_names + out_names
    if pid_name is not None:
        all_names = all_names + [pid_name]

    def _body(*args):
        operands = list(args)
        if pid_name is not None:
            operands.append(bass2jax.partition_id_tensor())
        outs = bass2jax._bass_exec_p.bind(
            *operands, out_avals=tuple(out_avals), in_names=tuple(all_names),
            out_names=tuple(out_names), lowering_input_output_aliases=(),
            sim_require_finite=True, sim_require_nnan=True, nc=nc)
        return tuple(outs)

    devices = jax.devices()[:N_CORES]
    mesh = Mesh(np.asarray(devices), ("core",))
    specs = (PartitionSpec("core"),) * (n_params + len(out_names))
    # No donation: the kernel writes every element of its outputs, so the
    # zero buffers can be passed persistently (lets the bench path reuse
    # device-resident buffers across calls).
    sharded = jax.jit(
        shard_map(_body, mesh=mesh, in_specs=specs,
                  out_specs=(PartitionSpec("core"),) * len(out_names),
                  check_rep=False),
        keep_unused=True)

    out_sharding = jax.sharding.NamedSharding(mesh, PartitionSpec("core"))

    def put(in_maps):
        """device_put the input slabs + persistent zero output buffers."""
        if not isinstance(in_maps, dict):
            in_maps = {n: np.concatenate([m[n] for m in in_maps], axis=0)
                       for n in in_maps[0]}
        dev_in = [jax.device_put(in_maps[n], out_sharding) for n in in_names]
        dev_zero = [jax.device_put(
            np.zeros((N_CORES * z.shape[0], *z.shape[1:]), z.dtype),
            out_sharding) for z in zero_outs]
        args = dev_in + dev_zero
        jax.block_until_ready(args)
        return args

    def exec_async(args):
        return sharded(*args)

    def run(in_maps):
        outs = exec_async(put(in_maps))
        return [{n: np.asarray(outs[i]).reshape(
                    N_CORES, *out_avals[i].shape)[c]
                 for i, n in enumerate(out_names)}
                for c in range(N_CORES)]

    run.put = put
    run.exec_async = exec_async
    return run


def _run_device(in_maps):
    if "runner" not in _CACHE:
        _CACHE["runner"] = _make_runner(_CACHE["nc"])
    return _CACHE["runner"](in_maps)


def _perm_gates(W):
    """PyTorch gate-row order [i,f,g,o] -> kernel order [i,f,o,g], with the
    g-gate rows scaled by 2 (tanh(x) = 2*sigmoid(2x) - 1 folding).

    W: (1024, ...) single-direction gate-stacked array."""
    return np.concatenate([W[0:512], W[768:1024], 2.0 * W[512:768]], axis=0)


def _sigmoid(x):
    return 1.0 / (1.0 + np.exp(-x))


def _lstm_dir_host(x, Wih, Whh, b):
    """Small (char) LSTM on host. x: (B,T,I) -> (B,T,Hd)."""
    xg = np.einsum('bti,gi->btg', x, Wih, optimize=True) + b
    xg = xg.astype(np.float32)
    Bs, Ts, G = xg.shape
    Hd = G // 4
    WhhT = np.ascontiguousarray(Whh.T)
    h = np.zeros((Bs, Hd), np.float32)
    c = np.zeros((Bs, Hd), np.float32)
    out = np.empty((Bs, Ts, Hd), np.float32)
    for t in range(Ts):
        g = xg[:, t] + h @ WhhT
        i = _sigmoid(g[:, :Hd])
        f = _sigmoid(g[:, Hd:2 * Hd])
        gg = np.tanh(g[:, 2 * Hd:3 * Hd])
        o = _sigmoid(g[:, 3 * Hd:])
        c = f * c + i * gg
        h = o * np.tanh(c)
        out[:, t] = h
    return out


def _logsumexp(a, axis):
    m = np.max(a, axis=axis, keepdims=True)
    return (m + np.log(np.sum(np.exp(a - m), axis=axis,
                              keepdims=True))).squeeze(axis)


def _pack_dir(x_grp, Wih_d, b_d, Whh_d, Wtag_rows, flip):
    """Build one core's in_map. x_grp: (16,T,320) f32."""
    import ml_dtypes
    f8np = ml_dtypes.float8_e4m3
    xs = x_grp[:, ::-1] if flip else x_grp
    xTm = np.empty((KD, NT), np.float32)
    # col = chain*2048 + t*8 + b: (2 chains, 8 seqs, T, 320) -> time-major
    xc = np.ascontiguousarray(xs).reshape(2, CW, T, EMB_IN)
    xc = xc.transpose(0, 2, 1, 3).reshape(2 * NC2, EMB_IN)  # (chain,t,b),f
    xTm[:320] = xc.T
    xTm[320] = 1.0
    wihm = np.empty((KD, 1024), np.float32)
    wihm[:320] = _perm_gates(Wih_d).T
    wihm[320] = _perm_gates(b_d.reshape(-1, 1))[:, 0]
    whhm = _perm_gates(Whh_d).T                      # (256, 1024)
    whh8 = np.ascontiguousarray(
        whhm.reshape(2, 128, 1024).transpose(1, 0, 2).reshape(128, 2048)
    ).astype(f8np)
    return {"xT": xTm.astype(f8np), "wih": wihm.astype(f8np), "whhT": whh8,
            "wtagT": np.ascontiguousarray(Wtag_rows).astype(f8np)}


def kernel(char_tensor, token_tensor, tags, mask, emb,
           cWih_f, cWhh_f, cb_f, cWih_b, cWhh_b, cb_b,
           wWih_f, wWhh_f, wb_f, wWih_b, wWhh_b, wb_b,
           Wtag, btag, start_t, end_t, trans):
    f32 = lambda a: np.asarray(a, np.float32)
    char_tensor = f32(char_tensor)
    emb = f32(emb)
    token_tensor = np.asarray(token_tensor).astype(np.int64)
    tags_i = np.asarray(tags).astype(np.int64)
    mask_b = np.asarray(mask).astype(bool)

    # --- char BiLSTM (tiny) + embedding gather on host ---
    cf = _lstm_dir_host(char_tensor, f32(cWih_f), f32(cWhh_f), f32(cb_f))
    cb = _lstm_dir_host(char_tensor[:, ::-1], f32(cWih_b), f32(cWhh_b),
                        f32(cb_b))[:, ::-1]
    word_emb = emb[token_tensor]                                  # (B,T,300)
    x = np.concatenate([cf, cb, word_emb], axis=2)                # (B,T,320)

    WtagT = np.ascontiguousarray(f32(Wtag).T)                     # (512, 20)
    per_core = []
    for g in range(4):
        xg_ = x[g * BL2:(g + 1) * BL2]
        per_core.append(_pack_dir(xg_, f32(wWih_f), f32(wb_f), f32(wWhh_f),
                                  WtagT[0:256], flip=False))
        per_core.append(_pack_dir(xg_, f32(wWih_b), f32(wb_b), f32(wWhh_b),
                                  WtagT[256:512], flip=True))
    # assemble the sharded slabs once here (host prep) so the device call
    # does no per-call concatenation
    in_maps = {n: np.concatenate([m[n] for m in per_core], axis=0)
               for n in per_core[0]}

    if "nc" not in _CACHE:
        _CACHE["nc"] = _build_nc()
    _CACHE["last_in_maps"] = in_maps
    # First exec on a freshly-compiled NEFF occasionally hits a transient
    # failure on this axon tunnel; retry (fresh build on second failure).
    res = None
    for attempt in range(3):
        try:
            res = _run_device(in_maps)
            break
        except Exception:
            if attempt == 2:
                raise
            import time as _time
            _time.sleep(5)
            _CACHE.pop("runner", None)
            if attempt == 1:
                _CACHE.pop("nc", None)
                _CACHE["nc"] = _build_nc()

    em = np.empty((B, T, K), np.float32)
    for g in range(4):
        ef = np.asarray(res[2 * g]["em"], np.float32)
        eb = np.asarray(res[2 * g + 1]["em"], np.float32)
        # col = chain*2048 + t*8 + b -> (chain,t,b) -> (16 seqs, T)
        ef = ef.T.reshape(2, T, CW, K).transpose(0, 2, 1, 3).reshape(
            BL2, T, K)
        eb = eb.T.reshape(2, T, CW, K).transpose(0, 2, 1, 3).reshape(
            BL2, T, K)[:, ::-1]                     # un-flip time
        em[g * BL2:(g + 1) * BL2] = ef + eb
    em += f32(btag)

    # --- CRF NLL on host (generic mask support) ---
    em = np.swapaxes(em, 0, 1)                                    # (T,B,K)
    tg = np.swapaxes(tags_i, 0, 1)
    m = np.swapaxes(mask_b, 0, 1).astype(np.float32)
    start_t, end_t, trans = f32(start_t), f32(end_t), f32(trans)
    bidx = np.arange(B)
    e_sc = np.take_along_axis(em, tg[..., None], axis=-1)[..., 0]  # (T,B)
    num = start_t[tg[0]] + e_sc[0]
    num = num + np.sum((trans[tg[:-1], tg[1:]] + e_sc[1:]) * m[1:], axis=0)
    last = (np.sum(m, axis=0) - 1).astype(np.int64)
    num = num + end_t[tg[last, bidx]]
    alpha = start_t[None, :] + em[0]
    for t in range(1, T):
        nxt = _logsumexp(alpha[:, :, None] + trans[None, :, :]
                         + em[t][:, None, :], axis=1)
        alpha = np.where(m[t][:, None] > 0, nxt, alpha)
    den = _logsumexp(alpha + end_t[None, :], axis=1)
    return np.float32(-np.sum(num - den))


# revision 7
# speedup vs baseline: 341.5810x; 1.0011x over previous
"""BiLSTM-CRF loss for nn_BiLSTM_CRF_68152541053203 on 8 TRN2 NeuronCores.

Sharding: batch x direction. B=64 splits into 4 groups of 16 sequences; each
group gets a core pair: core 2g runs the forward word-LSTM direction, core
2g+1 the backward direction (on host-time-flipped inputs, so the SPMD kernel
is identical). Each core computes on-device, SBUF-resident:
    xg = x @ Wih_d.T + b_d          (projected in 8-step chunks into PSUM)
    single-direction LSTM recurrence, 256 steps, fp8 DoubleRow matmuls
    em_part = seq_d @ Wtag_d.T      -> (20, 4096) fp8
Host: char BiLSTM + embedding gather (tiny), sums the two partial emissions
per group (+btag, bwd part time-unflipped), and runs the CRF forward scan
(generic mask support).

Key performance structure (v2):
  * Two independent 8-seq chains per core (A = seqs 0-7, B = 8-15),
    interleaved instruction-by-instruction so one chain's vector work hides
    the other's scalar-engine latency and cross-engine sync gaps.
  * The input projection xg = x@Wih.T runs on the (otherwise idle) PE in
    8-step chunks, directly into PSUM (start=True); the per-step Whh
    recurrence matmul then accumulates on top (start=False - PSUM has
    per-element has_written bits), so no separate gate-sum add is needed
    and the Activation engine reads gate pre-activations straight from
    PSUM. Chunks are double-buffered (4 PSUM banks total: chain x parity).
  * tanh(x) = 2*sigmoid(2x) - 1: the factor 2 is folded into the g-gate
    rows of Wih/Whh/bias on the host, so ONE sigmoid activation covers all
    four gates; a fused DVE tensor_scalar (2*s - 1) recovers tanh.

Device layouts (per core, chain width CW=8 seqs, T=256, chain cols NC2=2048):
  xT    (321, 4096) fp8e4m3: col = chain*2048 + t*8 + b (time-major within
        chain); rows 0:320 features, row 320 ones (bias row trick); bwd
        cores receive x time-reversed
  wih   (321, 1024) fp8e4m3: cols = this direction's gates, order [i,f,o,g],
        g-gate cols pre-scaled by 2
  whhT  (128, 2048) fp8e4m3: [p, k*1024+g] = WhhT_d[k*128+p, g] (DoubleRow),
        g-gate cols pre-scaled by 2
  wtagT (256, 20)  fp8e4m3: this direction's 256 rows of Wtag.T
  em    (20, 4096) fp8e4m3 output (partial emissions, no btag)
On-chip: sq (128, 2*4096) fp8 (col = k*4096 + chain*2048 + t*8 + b), which
doubles as the next step's matmul rhs and the emission matmul rhs;
gate/cell state f32 tiles per chain.
"""

import numpy as np

import concourse.bacc as bacc
import concourse.mybir as mybir
import concourse.tile as tile

N_CORES = 8
B, T = 64, 256
CIN, CH = 25, 10
EMB_IN, H = 320, 256
K = 20
BL2 = 16                      # sequences per core (4 groups x 2 dirs)
CW = 8                        # sequences per chain (2 chains per core)
NC2 = CW * T                  # 2048 cols per chain
NT = BL2 * T                  # 4096
KD = EMB_IN + 1
S = 8                         # steps per PSUM projection chunk
NG = T // S                   # 32 chunks
AF = mybir.ActivationFunctionType

_CACHE = {}


def _build_nc(repeat=1):
    from concourse.alu_op_type import AluOpType as ALU
    bf = mybir.dt.bfloat16
    f8 = mybir.dt.float8e4
    f32 = mybir.dt.float32
    nc = bacc.Bacc("TRN2", target_bir_lowering=False, debug=False,
                   num_devices=N_CORES)
    xT = nc.dram_tensor("xT", [KD, NT], f8, kind="ExternalInput").ap()
    wih = nc.dram_tensor("wih", [KD, 1024], f8, kind="ExternalInput").ap()
    whhT = nc.dram_tensor("whhT", [128, 2048], f8, kind="ExternalInput").ap()
    wtagT = nc.dram_tensor("wtagT", [H, K], f8, kind="ExternalInput").ap()
    em = nc.dram_tensor("em", [K, NT], f8, kind="ExternalOutput").ap()

    with tile.TileContext(nc) as tc:
        for _rep in range(repeat):
            _emit_body(nc, tc, xT, wih, whhT, wtagT, em, f8, f32, ALU)
    nc.compile()
    return nc


def _emit_body(nc, tc, xT, wih, whhT, wtagT, em, f8, f32, ALU):
    DR = mybir.MatmulPerfMode.DoubleRow
    with (
        tc.tile_pool(name="per", bufs=1) as per,
        tc.tile_pool(name="st", bufs=2) as st,
    ):
        wih01 = per.tile([128, 2 * 1024], f8, tag="wih01", name="wih01")
        nc.gpsimd.dma_start(wih01[:, 0:1024], wih[0:128, :])
        nc.gpsimd.dma_start(wih01[:, 1024:2048], wih[128:256, :])
        wih01r = wih01[:].rearrange("p (k g) -> p k g", k=2)
        wih2 = per.tile([65, 1024], f8, tag="wih2", name="wih2")
        nc.gpsimd.dma_start(wih2[:], wih[256:321, :])
        xT01 = per.tile([128, 2 * NT], f8, tag="xT01", name="xT01")
        nc.gpsimd.dma_start(xT01[:, 0:NT], xT[0:128, :])
        nc.gpsimd.dma_start(xT01[:, NT:2 * NT], xT[128:256, :])
        xT01r = xT01[:].rearrange("p (k n) -> p k n", k=2)
        xT2 = per.tile([65, NT], f8, tag="xT2", name="xT2")
        nc.gpsimd.dma_start(xT2[:], xT[256:321, :])
        whh8 = per.tile([128, 2048], f8, tag="whh8", name="whh8")
        nc.gpsimd.dma_start(whh8[:], whhT[:, :])
        whh8r = whh8[:].rearrange("p (k g) -> p k g", k=2)
        wt8 = per.tile([128, 2 * K], f8, tag="wt8", name="wt8")
        nc.gpsimd.dma_start(wt8[:, 0:K], wtagT[0:128, :])
        nc.gpsimd.dma_start(wt8[:, K:2 * K], wtagT[128:256, :])
        wt8r = wt8[:].rearrange("p (k n) -> p k n", k=2)

        # h history; col = k*4096 + chain*2048 + t*8 + b
        sq = per.tile([128, 2 * NT], f8, tag="sq", name="sq")
        sq4 = sq[:].rearrange("p (k c n) -> p k c n", k=2, c=2)
        # per-chain gate tiles: [i 0:16 | f 16:32 | o 32:48 | g 48:64]
        ga2 = per.tile([128, 128], f32, tag="ga2", name="ga2")
        # per-chain [ghat 0:16 | c 16:32]
        gc2 = per.tile([128, 64], f32, tag="gc2", name="gc2")
        th2 = per.tile([128, 32], f32, tag="th2", name="th2")
        nc.vector.memset(gc2[:, 16:32], 0.0)
        nc.vector.memset(gc2[:, 48:64], 0.0)

        with tc.tile_pool(name="prm", bufs=1, space="PSUM") as prm:
            # 4 separate one-bank tiles: q = chain*2 + (group parity); each
            # holds an 8-step chunk of gate pre-activations
            # [c*64 + t_rel*8 + b]. Separate tiles (not one 4-bank tile) so
            # tile-granular dependency tracking doesn't serialize one
            # chain's activation behind the other chain's / next chunk's
            # projection matmuls.
            psq = [prm.tile([128, 512], f32, tag=f"ps{q}", name=f"ps{q}")
                   for q in range(4)]
            psvq = [p[:] for p in psq]
            psrq = [p[:].rearrange("p (c n) -> p c n", c=8) for p in psq]

            def proj(chain, g, c):
                """Project xg for chunk g (8 steps), gate-chunk c, into
                PSUM bank chain*2 + g%2."""
                q = chain * 2 + (g % 2)
                dst = psvq[q][:, c * 64:(c + 1) * 64]
                cols = slice(chain * NC2 + g * 64, chain * NC2 + (g + 1) * 64)
                nc.tensor.matmul(dst, wih01r[:, :, c * 128:(c + 1) * 128],
                                 xT01r[:, :, cols], perf_mode=DR,
                                 start=True, stop=False)
                nc.tensor.matmul(dst, wih2[:, c * 128:(c + 1) * 128],
                                 xT2[:, cols], start=False, stop=True)

            for c in range(8):
                proj(0, 0, c)
                proj(1, 0, c)

            for g in range(NG):
                for tr in range(S):
                    t = g * S + tr
                    q0, q1 = 0 * 2 + (g % 2), 1 * 2 + (g % 2)
                    # --- PE: recurrence matmuls accumulate onto xg ---
                    if t > 0:
                        # chunk-major: chain A and B matmuls for the same
                        # chunk are adjacent, so the second one reuses the
                        # just-loaded Whh chunk weights (PE background
                        # weight buffer hides the reload).
                        for c in range(8):
                            for chain, q in ((0, q0), (1, q1)):
                                nc.tensor.matmul(
                                    psvq[q][:, c * 64 + tr * 8:
                                            c * 64 + tr * 8 + 8],
                                    whh8r[:, :, c * 128:(c + 1) * 128],
                                    sq4[:, :, chain, (t - 1) * 8:t * 8],
                                    perf_mode=DR, start=False, stop=True,
                                    skip_group_check=True)
                    # --- PE: next chunk's projection, spread over steps ---
                    if g + 1 < NG:
                        proj(0, g + 1, tr)
                        proj(1, g + 1, tr)
                    # --- Act: one sigmoid over all gates (2x folded in g) ---
                    for chain, q in ((0, q0), (1, q1)):
                        nc.scalar.activation(
                            ga2[:, chain * 64:chain * 64 + 64],
                            psrq[q][:, :, tr * 8:(tr + 1) * 8], AF.Sigmoid)
                    # --- DVE chain A, then Act tanh A, then DVE chain B ---
                    for chain in (0, 1):
                        CB = chain * 64
                        GB = chain * 32
                        gh = gc2[:, GB:GB + 16]
                        cc = gc2[:, GB + 16:GB + 32]
                        nc.vector.tensor_scalar(
                            gh, ga2[:, CB + 48:CB + 64], 2.0, -1.0,
                            ALU.mult, ALU.add)
                        t12 = st.tile([128, 32], f32, tag=f"t12{chain}",
                                      name=f"t12{chain}")
                        nc.vector.tensor_mul(t12[:], ga2[:, CB:CB + 32],
                                             gc2[:, GB:GB + 32])
                        nc.vector.tensor_add(cc, t12[:, 0:16], t12[:, 16:32])
                        nc.scalar.activation(th2[:, GB // 2:GB // 2 + 16],
                                             cc, AF.Tanh)
                    for chain in (0, 1):
                        CB = chain * 64
                        nc.vector.tensor_mul(
                            sq4[:, :, chain, t * 8:(t + 1) * 8],
                            ga2[:, CB + 32:CB + 48],
                            th2[:, chain * 16:chain * 16 + 16])

        # partial emissions: em = seq_d @ Wtag_d.T (DoubleRow over k)
        em_sb = per.tile([K, NT], f8, tag="em", name="em")
        with tc.tile_pool(name="pe", bufs=2, space="PSUM") as pe:
            for chain in range(2):
                for nb in range(4):
                    ps = pe.tile([K, 512], f32, tag="eps", name="eps")
                    for k in range(2):
                        nc.tensor.matmul(
                            ps[:], wt8r[:, k, :],
                            sq4[:, k, chain, nb * 512:(nb + 1) * 512],
                            start=(k == 0), stop=(k == 1))
                    dst = em_sb[:, chain * NC2 + nb * 512:
                                chain * NC2 + (nb + 1) * 512]
                    if nb % 2 == 0:
                        nc.vector.tensor_copy(dst, ps[:])
                    else:
                        nc.scalar.copy(dst, ps[:])
            nc.gpsimd.dma_start(em[:, :], em_sb[:])


def _make_runner(nc):
    """Build the jitted PJRT executor once (run_bass_via_pjrt re-traces on
    every call; caching the jit + shard_map saves that per-call cost)."""
    import jax
    from jax.sharding import Mesh, PartitionSpec
    from jax.experimental.shard_map import shard_map
    from concourse import bass2jax
    bass2jax.install_neuronx_cc_hook()
    assert nc.dbg_addr is None
    pid_name = (nc.partition_id_tensor.name
                if nc.partition_id_tensor else None)

    in_names, out_names, out_avals, zero_outs = [], [], [], []
    for alloc in nc.m.functions[0].allocations:
        if not isinstance(alloc, mybir.MemoryLocationSet):
            continue
        name = alloc.memorylocations[0].name
        if alloc.kind == "ExternalInput":
            if name != pid_name:
                in_names.append(name)
        elif alloc.kind == "ExternalOutput":
            out_names.append(name)
            shape = tuple(alloc.tensor_shape)
            dtype = mybir.dt.np(alloc.dtype)
            out_avals.append(jax.core.ShapedArray(shape, dtype))
            zero_outs.append(np.zeros(shape, dtype))
    n_params = len(in_names)
    all_names = in_names + out_names
    if pid_name is not None:
        all_names = all_names + [pid_name]

    def _body(*args):
        operands = list(args)
        if pid_name is not None:
            operands.append(bass2jax.partition_id_tensor())
        outs = bass2jax._bass_exec_p.bind(
            *operands, out_avals=tuple(out_avals), in_names=tuple(all_names),
            out_names=tuple(out_names), lowering_input_output_aliases=(),
            sim_require_finite=True, sim_require_nnan=True, nc=nc)
        return tuple(outs)

    devices = jax.devices()[:N_CORES]
    mesh = Mesh(np.asarray(devices), ("core",))
    specs = (PartitionSpec("core"),) * (n_params + len(out_names))
    # No donation: the kernel writes every element of its outputs, so the
    # zero buffers can be passed persistently (lets the bench path reuse
    # device-resident buffers across calls).
    sharded = jax.jit(
        shard_map(_body, mesh=mesh, in_specs=specs,
                  out_specs=(PartitionSpec("core"),) * len(out_names),
                  check_rep=False),
        keep_unused=True)

    out_sharding = jax.sharding.NamedSharding(mesh, PartitionSpec("core"))

    def put(in_maps):
        """device_put the input slabs + persistent zero output buffers."""
        if not isinstance(in_maps, dict):
            in_maps = {n: np.concatenate([m[n] for m in in_maps], axis=0)
                       for n in in_maps[0]}
        dev_in = [jax.device_put(in_maps[n], out_sharding) for n in in_names]
        dev_zero = [jax.device_put(
            np.zeros((N_CORES * z.shape[0], *z.shape[1:]), z.dtype),
            out_sharding) for z in zero_outs]
        args = dev_in + dev_zero
        jax.block_until_ready(args)
        return args

    def exec_async(args):
        return sharded(*args)

    def run(in_maps):
        outs = exec_async(put(in_maps))
        return [{n: np.asarray(outs[i]).reshape(
                    N_CORES, *out_avals[i].shape)[c]
                 for i, n in enumerate(out_names)}
                for c in range(N_CORES)]

    run.put = put
    run.exec_async = exec_async
    return run


def _run_device(in_maps):
    if "runner" not in _CACHE:
        _CACHE["runner"] = _make_runner(_CACHE["nc"])
    return _CACHE["runner"](in_maps)


def _perm_gates(W):
    """PyTorch gate-row order [i,f,g,o] -> kernel order [i,f,o,g], with the
    g-gate rows scaled by 2 (tanh(x) = 2*sigmoid(2x) - 1 folding).

    W: (1024, ...) single-direction gate-stacked array."""
    return np.concatenate([W[0:512], W[768:1024], 2.0 * W[512:768]], axis=0)


def _sigmoid(x):
    return 1.0 / (1.0 + np.exp(-x))


def _lstm_dir_host(x, Wih, Whh, b):
    """Small (char) LSTM on host. x: (B,T,I) -> (B,T,Hd)."""
    xg = np.einsum('bti,gi->btg', x, Wih, optimize=True) + b
    xg = xg.astype(np.float32)
    Bs, Ts, G = xg.shape
    Hd = G // 4
    WhhT = np.ascontiguousarray(Whh.T)
    h = np.zeros((Bs, Hd), np.float32)
    c = np.zeros((Bs, Hd), np.float32)
    out = np.empty((Bs, Ts, Hd), np.float32)
    for t in range(Ts):
        g = xg[:, t] + h @ WhhT
        i = _sigmoid(g[:, :Hd])
        f = _sigmoid(g[:, Hd:2 * Hd])
        gg = np.tanh(g[:, 2 * Hd:3 * Hd])
        o = _sigmoid(g[:, 3 * Hd:])
        c = f * c + i * gg
        h = o * np.tanh(c)
        out[:, t] = h
    return out


def _logsumexp(a, axis):
    m = np.max(a, axis=axis, keepdims=True)
    return (m + np.log(np.sum(np.exp(a - m), axis=axis,
                              keepdims=True))).squeeze(axis)


def _pack_dir(x_grp, Wih_d, b_d, Whh_d, Wtag_rows, flip):
    """Build one core's in_map. x_grp: (16,T,320) f32."""
    import ml_dtypes
    f8np = ml_dtypes.float8_e4m3
    xs = x_grp[:, ::-1] if flip else x_grp
    xTm = np.empty((KD, NT), np.float32)
    # col = chain*2048 + t*8 + b: (2 chains, 8 seqs, T, 320) -> time-major
    xc = np.ascontiguousarray(xs).reshape(2, CW, T, EMB_IN)
    xc = xc.transpose(0, 2, 1, 3).reshape(2 * NC2, EMB_IN)  # (chain,t,b),f
    xTm[:320] = xc.T
    xTm[320] = 1.0
    wihm = np.empty((KD, 1024), np.float32)
    wihm[:320] = _perm_gates(Wih_d).T
    wihm[320] = _perm_gates(b_d.reshape(-1, 1))[:, 0]
    whhm = _perm_gates(Whh_d).T                      # (256, 1024)
    whh8 = np.ascontiguousarray(
        whhm.reshape(2, 128, 1024).transpose(1, 0, 2).reshape(128, 2048)
    ).astype(f8np)
    return {"xT": xTm.astype(f8np), "wih": wihm.astype(f8np), "whhT": whh8,
            "wtagT": np.ascontiguousarray(Wtag_rows).astype(f8np)}


def kernel(char_tensor, token_tensor, tags, mask, emb,
           cWih_f, cWhh_f, cb_f, cWih_b, cWhh_b, cb_b,
           wWih_f, wWhh_f, wb_f, wWih_b, wWhh_b, wb_b,
           Wtag, btag, start_t, end_t, trans):
    f32 = lambda a: np.asarray(a, np.float32)
    char_tensor = f32(char_tensor)
    emb = f32(emb)
    token_tensor = np.asarray(token_tensor).astype(np.int64)
    tags_i = np.asarray(tags).astype(np.int64)
    mask_b = np.asarray(mask).astype(bool)

    # --- char BiLSTM (tiny) + embedding gather on host ---
    cf = _lstm_dir_host(char_tensor, f32(cWih_f), f32(cWhh_f), f32(cb_f))
    cb = _lstm_dir_host(char_tensor[:, ::-1], f32(cWih_b), f32(cWhh_b),
                        f32(cb_b))[:, ::-1]
    word_emb = emb[token_tensor]                                  # (B,T,300)
    x = np.concatenate([cf, cb, word_emb], axis=2)                # (B,T,320)

    WtagT = np.ascontiguousarray(f32(Wtag).T)                     # (512, 20)
    per_core = []
    for g in range(4):
        xg_ = x[g * BL2:(g + 1) * BL2]
        per_core.append(_pack_dir(xg_, f32(wWih_f), f32(wb_f), f32(wWhh_f),
                                  WtagT[0:256], flip=False))
        per_core.append(_pack_dir(xg_, f32(wWih_b), f32(wb_b), f32(wWhh_b),
                                  WtagT[256:512], flip=True))
    # assemble the sharded slabs once here (host prep) so the device call
    # does no per-call concatenation
    in_maps = {n: np.concatenate([m[n] for m in per_core], axis=0)
               for n in per_core[0]}

    if "nc" not in _CACHE:
        _CACHE["nc"] = _build_nc()
    _CACHE["last_in_maps"] = in_maps
    # First exec on a freshly-compiled NEFF occasionally hits a transient
    # failure on this axon tunnel; retry (fresh build on second failure).
    res = None
    for attempt in range(3):
        try:
            res = _run_device(in_maps)
            break
        except Exception:
            if attempt == 2:
                raise
            import time as _time
            _time.sleep(5)
            _CACHE.pop("runner", None)
            if attempt == 1:
                _CACHE.pop("nc", None)
                _CACHE["nc"] = _build_nc()

    em = np.empty((B, T, K), np.float32)
    for g in range(4):
        ef = np.asarray(res[2 * g]["em"], np.float32)
        eb = np.asarray(res[2 * g + 1]["em"], np.float32)
        # col = chain*2048 + t*8 + b -> (chain,t,b) -> (16 seqs, T)
        ef = ef.T.reshape(2, T, CW, K).transpose(0, 2, 1, 3).reshape(
            BL2, T, K)
        eb = eb.T.reshape(2, T, CW, K).transpose(0, 2, 1, 3).reshape(
            BL2, T, K)[:, ::-1]                     # un-flip time
        em[g * BL2:(g + 1) * BL2] = ef + eb
    em += f32(btag)

    # --- CRF NLL on host (generic mask support) ---
    em = np.swapaxes(em, 0, 1)                                    # (T,B,K)
    tg = np.swapaxes(tags_i, 0, 1)
    m = np.swapaxes(mask_b, 0, 1).astype(np.float32)
    start_t, end_t, trans = f32(start_t), f32(end_t), f32(trans)
    bidx = np.arange(B)
    e_sc = np.take_along_axis(em, tg[..., None], axis=-1)[..., 0]  # (T,B)
    num = start_t[tg[0]] + e_sc[0]
    num = num + np.sum((trans[tg[:-1], tg[1:]] + e_sc[1:]) * m[1:], axis=0)
    last = (np.sum(m, axis=0) - 1).astype(np.int64)
    num = num + end_t[tg[last, bidx]]
    alpha = start_t[None, :] + em[0]
    for t in range(1, T):
        nxt = _logsumexp(alpha[:, :, None] + trans[None, :, :]
                         + em[t][:, None, :], axis=1)
        alpha = np.where(m[t][:, None] > 0, nxt, alpha)
    den = _logsumexp(alpha + end_t[None, :], axis=1)
    return np.float32(-np.sum(num - den))


# revision 11
# speedup vs baseline: 358.3642x; 1.0491x over previous
"""BiLSTM-CRF loss for nn_BiLSTM_CRF_68152541053203 on 8 TRN2 NeuronCores.

Sharding: batch x direction. B=64 splits into 4 groups of 16 sequences; each
group gets a core pair: core 2g runs the forward word-LSTM direction, core
2g+1 the backward direction (on host-time-flipped inputs, so the SPMD kernel
is identical). Each core computes on-device, SBUF-resident:
    xg = x @ Wih_d.T + b_d          (projected in 8-step chunks into PSUM)
    single-direction LSTM recurrence, 256 steps, fp8 DoubleRow matmuls
    em_part = seq_d @ Wtag_d.T      -> (20, 4096) fp8
Host: char BiLSTM + embedding gather (tiny), sums the two partial emissions
per group (+btag, bwd part time-unflipped), and runs the CRF forward scan
(generic mask support).

Key performance structure (v2):
  * Two independent 8-seq chains per core (A = seqs 0-7, B = 8-15),
    interleaved instruction-by-instruction so one chain's vector work hides
    the other's scalar-engine latency and cross-engine sync gaps.
  * The input projection xg = x@Wih.T runs on the (otherwise idle) PE in
    8-step chunks, directly into PSUM (start=True); the per-step Whh
    recurrence matmul then accumulates on top (start=False - PSUM has
    per-element has_written bits), so no separate gate-sum add is needed
    and the Activation engine reads gate pre-activations straight from
    PSUM. Chunks are double-buffered (4 PSUM banks total: chain x parity).
  * tanh(x) = 2*sigmoid(2x) - 1: the factor 2 is folded into the g-gate
    rows of Wih/Whh/bias on the host, so ONE sigmoid activation covers all
    four gates; a fused DVE tensor_scalar (2*s - 1) recovers tanh.

Device layouts (per core, chain width CW=8 seqs, T=256, chain cols NC2=2048):
  xT    (321, 4096) fp8e4m3: col = chain*2048 + t*8 + b (time-major within
        chain); rows 0:320 features, row 320 ones (bias row trick); bwd
        cores receive x time-reversed
  wih   (321, 1024) fp8e4m3: cols = this direction's gates, order [i,f,o,g],
        g-gate cols pre-scaled by 2
  whhT  (128, 2048) fp8e4m3: [p, k*1024+g] = WhhT_d[k*128+p, g] (DoubleRow),
        g-gate cols pre-scaled by 2
  wtagT (256, 20)  fp8e4m3: this direction's 256 rows of Wtag.T
  em    (20, 4096) fp8e4m3 output (partial emissions, no btag)
On-chip: sq (128, 2*4096) fp8 (col = k*4096 + chain*2048 + t*8 + b), which
doubles as the next step's matmul rhs and the emission matmul rhs;
gate/cell state f32 tiles per chain.
"""

import numpy as np

import concourse.bacc as bacc
import concourse.mybir as mybir
import concourse.tile as tile

N_CORES = 8
B, T = 64, 256
CIN, CH = 25, 10
EMB_IN, H = 320, 256
K = 20
BL2 = 16                      # sequences per core (4 groups x 2 dirs)
CW = 8                        # sequences per chain (2 chains per core)
NC2 = CW * T                  # 2048 cols per chain
NT = BL2 * T                  # 4096
KD = EMB_IN + 1
S = 8                         # steps per PSUM projection chunk
NG = T // S                   # 32 chunks
AF = mybir.ActivationFunctionType

_CACHE = {}


def _build_nc(repeat=1):
    from concourse.alu_op_type import AluOpType as ALU
    bf = mybir.dt.bfloat16
    f8 = mybir.dt.float8e4
    f32 = mybir.dt.float32
    nc = bacc.Bacc("TRN2", target_bir_lowering=False, debug=False,
                   num_devices=N_CORES)
    xT = nc.dram_tensor("xT", [KD, NT], f8, kind="ExternalInput").ap()
    wih = nc.dram_tensor("wih", [KD, 1024], f8, kind="ExternalInput").ap()
    whhT = nc.dram_tensor("whhT", [128, 2048], f8, kind="ExternalInput").ap()
    wtagT = nc.dram_tensor("wtagT", [H, K], f8, kind="ExternalInput").ap()
    em = nc.dram_tensor("em", [K, NT], f8, kind="ExternalOutput").ap()

    with tile.TileContext(nc) as tc:
        for _rep in range(repeat):
            _emit_body(nc, tc, xT, wih, whhT, wtagT, em, f8, f32, ALU)
    nc.compile()
    return nc


def _emit_body(nc, tc, xT, wih, whhT, wtagT, em, f8, f32, ALU):
    DR = mybir.MatmulPerfMode.DoubleRow
    with (
        tc.tile_pool(name="per", bufs=1) as per,
        tc.tile_pool(name="st", bufs=2) as st,
    ):
        wih01 = per.tile([128, 2 * 1024], f8, tag="wih01", name="wih01")
        nc.gpsimd.dma_start(wih01[:, 0:1024], wih[0:128, :])
        nc.gpsimd.dma_start(wih01[:, 1024:2048], wih[128:256, :])
        wih01r = wih01[:].rearrange("p (k g) -> p k g", k=2)
        wih2 = per.tile([65, 1024], f8, tag="wih2", name="wih2")
        nc.gpsimd.dma_start(wih2[:], wih[256:321, :])
        xT01 = per.tile([128, 2 * NT], f8, tag="xT01", name="xT01")
        nc.gpsimd.dma_start(xT01[:, 0:NT], xT[0:128, :])
        nc.gpsimd.dma_start(xT01[:, NT:2 * NT], xT[128:256, :])
        xT01r = xT01[:].rearrange("p (k n) -> p k n", k=2)
        xT2 = per.tile([65, NT], f8, tag="xT2", name="xT2")
        nc.gpsimd.dma_start(xT2[:], xT[256:321, :])
        whh8 = per.tile([128, 2048], f8, tag="whh8", name="whh8")
        nc.gpsimd.dma_start(whh8[:], whhT[:, :])
        whh8r = whh8[:].rearrange("p (k g) -> p k g", k=2)
        wt8 = per.tile([128, 2 * K], f8, tag="wt8", name="wt8")
        nc.gpsimd.dma_start(wt8[:, 0:K], wtagT[0:128, :])
        nc.gpsimd.dma_start(wt8[:, K:2 * K], wtagT[128:256, :])
        wt8r = wt8[:].rearrange("p (k n) -> p k n", k=2)

        # h history; col = k*4096 + chain*2048 + t*8 + b
        sq = per.tile([128, 2 * NT], f8, tag="sq", name="sq")
        sq4 = sq[:].rearrange("p (k c n) -> p k c n", k=2, c=2)
        # per-chain gate tiles: [i 0:16 | f 16:32 | o 32:48 | g 48:64]
        ga2 = per.tile([128, 128], f32, tag="ga2", name="ga2")
        # per-chain [ghat 0:16 | c 16:32]
        gc2 = per.tile([128, 64], f32, tag="gc2", name="gc2")
        th2 = per.tile([128, 32], f32, tag="th2", name="th2")
        nc.vector.memset(gc2[:, 16:32], 0.0)
        nc.vector.memset(gc2[:, 48:64], 0.0)

        with tc.tile_pool(name="prm", bufs=1, space="PSUM") as prm:
            # 4 separate one-bank tiles: q = chain*2 + (group parity); each
            # holds an 8-step chunk of gate pre-activations
            # [c*64 + t_rel*8 + b]. Separate tiles (not one 4-bank tile) so
            # tile-granular dependency tracking doesn't serialize one
            # chain's activation behind the other chain's / next chunk's
            # projection matmuls.
            psq = [prm.tile([128, 512], f32, tag=f"ps{q}", name=f"ps{q}")
                   for q in range(4)]
            psvq = [p[:] for p in psq]
            psrq = [p[:].rearrange("p (c n) -> p c n", c=8) for p in psq]

            def proj(chain, g, c):
                """Project xg for chunk g (8 steps), gate-chunk c, into
                PSUM bank chain*2 + g%2."""
                q = chain * 2 + (g % 2)
                dst = psvq[q][:, c * 64:(c + 1) * 64]
                cols = slice(chain * NC2 + g * 64, chain * NC2 + (g + 1) * 64)
                nc.tensor.matmul(dst, wih01r[:, :, c * 128:(c + 1) * 128],
                                 xT01r[:, :, cols], perf_mode=DR,
                                 start=True, stop=False)
                nc.tensor.matmul(dst, wih2[:, c * 128:(c + 1) * 128],
                                 xT2[:, cols], start=False, stop=True)

            for c in range(8):
                proj(0, 0, c)
                proj(1, 0, c)

            for g in range(NG):
                for tr in range(S):
                    t = g * S + tr
                    q0, q1 = 0 * 2 + (g % 2), 1 * 2 + (g % 2)
                    # --- PE: recurrence matmuls accumulate onto xg ---
                    if t > 0:
                        # chunk-major: chain A and B matmuls for the same
                        # chunk are adjacent, so the second one reuses the
                        # just-loaded Whh chunk weights (PE background
                        # weight buffer hides the reload).
                        for c in range(8):
                            for chain, q in ((0, q0), (1, q1)):
                                nc.tensor.matmul(
                                    psvq[q][:, c * 64 + tr * 8:
                                            c * 64 + tr * 8 + 8],
                                    whh8r[:, :, c * 128:(c + 1) * 128],
                                    sq4[:, :, chain, (t - 1) * 8:t * 8],
                                    perf_mode=DR, start=False, stop=True,
                                    skip_group_check=True)
                    # --- PE: next chunk's projection, spread over steps ---
                    if g + 1 < NG:
                        proj(0, g + 1, tr)
                        proj(1, g + 1, tr)
                    # --- Act: one sigmoid over all gates (2x folded in g) ---
                    for chain, q in ((0, q0), (1, q1)):
                        nc.scalar.activation(
                            ga2[:, chain * 64:chain * 64 + 64],
                            psrq[q][:, :, tr * 8:(tr + 1) * 8], AF.Sigmoid)
                    # --- DVE chain A, then Act tanh A, then DVE chain B ---
                    # cell state is kept pre-halved (ct = c/2):
                    #   ct' = f*ct + (sg - 1/2)*i   [= c'/2]
                    #   tanh(c') = tanh(2*ct')      [free via Act pre-scale]
                    # so the two products depend only on the sigmoid output
                    # (no serial ghat->mul link) and the 2s-1 fixup vanishes.
                    for chain in (0, 1):
                        CB = chain * 64
                        GB = chain * 32
                        cc = gc2[:, GB + 16:GB + 32]
                        t12 = st.tile([128, 32], f32, tag=f"t12{chain}",
                                      name=f"t12{chain}")
                        nc.vector.tensor_mul(t12[:, 16:32],
                                             ga2[:, CB + 16:CB + 32], cc)
                        nc.vector.scalar_tensor_tensor(
                            t12[:, 0:16], ga2[:, CB + 48:CB + 64], -0.5,
                            ga2[:, CB:CB + 16], ALU.add, ALU.mult)
                        nc.vector.tensor_add(cc, t12[:, 0:16], t12[:, 16:32])
                        nc.scalar.activation(th2[:, GB // 2:GB // 2 + 16],
                                             cc, AF.Tanh, scale=2.0)
                    for chain in (0, 1):
                        CB = chain * 64
                        nc.vector.tensor_mul(
                            sq4[:, :, chain, t * 8:(t + 1) * 8],
                            ga2[:, CB + 32:CB + 48],
                            th2[:, chain * 16:chain * 16 + 16])

        # partial emissions: em = seq_d @ Wtag_d.T (DoubleRow over k)
        em_sb = per.tile([K, NT], f8, tag="em", name="em")
        with tc.tile_pool(name="pe", bufs=2, space="PSUM") as pe:
            for chain in range(2):
                for nb in range(4):
                    ps = pe.tile([K, 512], f32, tag="eps", name="eps")
                    for k in range(2):
                        nc.tensor.matmul(
                            ps[:], wt8r[:, k, :],
                            sq4[:, k, chain, nb * 512:(nb + 1) * 512],
                            start=(k == 0), stop=(k == 1))
                    dst = em_sb[:, chain * NC2 + nb * 512:
                                chain * NC2 + (nb + 1) * 512]
                    if nb % 2 == 0:
                        nc.vector.tensor_copy(dst, ps[:])
                    else:
                        nc.scalar.copy(dst, ps[:])
            nc.gpsimd.dma_start(em[:, :], em_sb[:])


def _make_runner(nc):
    """Build the jitted PJRT executor once (run_bass_via_pjrt re-traces on
    every call; caching the jit + shard_map saves that per-call cost)."""
    import jax
    from jax.sharding import Mesh, PartitionSpec
    from jax.experimental.shard_map import shard_map
    from concourse import bass2jax
    bass2jax.install_neuronx_cc_hook()
    assert nc.dbg_addr is None
    pid_name = (nc.partition_id_tensor.name
                if nc.partition_id_tensor else None)

    in_names, out_names, out_avals, zero_outs = [], [], [], []
    for alloc in nc.m.functions[0].allocations:
        if not isinstance(alloc, mybir.MemoryLocationSet):
            continue
        name = alloc.memorylocations[0].name
        if alloc.kind == "ExternalInput":
            if name != pid_name:
                in_names.append(name)
        elif alloc.kind == "ExternalOutput":
            out_names.append(name)
            shape = tuple(alloc.tensor_shape)
            dtype = mybir.dt.np(alloc.dtype)
            out_avals.append(jax.core.ShapedArray(shape, dtype))
            zero_outs.append(np.zeros(shape, dtype))
    n_params = len(in_names)
    all_names = in_names + out_names
    if pid_name is not None:
        all_names = all_names + [pid_name]

    def _body(*args):
        operands = list(args)
        if pid_name is not None:
            operands.append(bass2jax.partition_id_tensor())
        outs = bass2jax._bass_exec_p.bind(
            *operands, out_avals=tuple(out_avals), in_names=tuple(all_names),
            out_names=tuple(out_names), lowering_input_output_aliases=(),
            sim_require_finite=True, sim_require_nnan=True, nc=nc)
        return tuple(outs)

    devices = jax.devices()[:N_CORES]
    mesh = Mesh(np.asarray(devices), ("core",))
    specs = (PartitionSpec("core"),) * (n_params + len(out_names))
    # No donation: the kernel writes every element of its outputs, so the
    # zero buffers can be passed persistently (lets the bench path reuse
    # device-resident buffers across calls).
    sharded = jax.jit(
        shard_map(_body, mesh=mesh, in_specs=specs,
                  out_specs=(PartitionSpec("core"),) * len(out_names),
                  check_rep=False),
        keep_unused=True)

    out_sharding = jax.sharding.NamedSharding(mesh, PartitionSpec("core"))

    def put(in_maps):
        """device_put the input slabs + persistent zero output buffers."""
        if not isinstance(in_maps, dict):
            in_maps = {n: np.concatenate([m[n] for m in in_maps], axis=0)
                       for n in in_maps[0]}
        dev_in = [jax.device_put(in_maps[n], out_sharding) for n in in_names]
        dev_zero = [jax.device_put(
            np.zeros((N_CORES * z.shape[0], *z.shape[1:]), z.dtype),
            out_sharding) for z in zero_outs]
        args = dev_in + dev_zero
        jax.block_until_ready(args)
        return args

    def exec_async(args):
        return sharded(*args)

    def run(in_maps):
        outs = exec_async(put(in_maps))
        return [{n: np.asarray(outs[i]).reshape(
                    N_CORES, *out_avals[i].shape)[c]
                 for i, n in enumerate(out_names)}
                for c in range(N_CORES)]

    run.put = put
    run.exec_async = exec_async
    return run


def _run_device(in_maps):
    if "runner" not in _CACHE:
        _CACHE["runner"] = _make_runner(_CACHE["nc"])
    return _CACHE["runner"](in_maps)


def _perm_gates(W):
    """PyTorch gate-row order [i,f,g,o] -> kernel order [i,f,o,g], with the
    g-gate rows scaled by 2 (tanh(x) = 2*sigmoid(2x) - 1 folding).

    W: (1024, ...) single-direction gate-stacked array."""
    return np.concatenate([W[0:512], W[768:1024], 2.0 * W[512:768]], axis=0)


def _sigmoid(x):
    return 1.0 / (1.0 + np.exp(-x))


def _lstm_dir_host(x, Wih, Whh, b):
    """Small (char) LSTM on host. x: (B,T,I) -> (B,T,Hd)."""
    xg = np.einsum('bti,gi->btg', x, Wih, optimize=True) + b
    xg = xg.astype(np.float32)
    Bs, Ts, G = xg.shape
    Hd = G // 4
    WhhT = np.ascontiguousarray(Whh.T)
    h = np.zeros((Bs, Hd), np.float32)
    c = np.zeros((Bs, Hd), np.float32)
    out = np.empty((Bs, Ts, Hd), np.float32)
    for t in range(Ts):
        g = xg[:, t] + h @ WhhT
        i = _sigmoid(g[:, :Hd])
        f = _sigmoid(g[:, Hd:2 * Hd])
        gg = np.tanh(g[:, 2 * Hd:3 * Hd])
        o = _sigmoid(g[:, 3 * Hd:])
        c = f * c + i * gg
        h = o * np.tanh(c)
        out[:, t] = h
    return out


def _logsumexp(a, axis):
    m = np.max(a, axis=axis, keepdims=True)
    return (m + np.log(np.sum(np.exp(a - m), axis=axis,
                              keepdims=True))).squeeze(axis)


def _pack_dir(x_grp, Wih_d, b_d, Whh_d, Wtag_rows, flip):
    """Build one core's in_map. x_grp: (16,T,320) f32."""
    import ml_dtypes
    f8np = ml_dtypes.float8_e4m3
    xs = x_grp[:, ::-1] if flip else x_grp
    xTm = np.empty((KD, NT), np.float32)
    # col = chain*2048 + t*8 + b: (2 chains, 8 seqs, T, 320) -> time-major
    xc = np.ascontiguousarray(xs).reshape(2, CW, T, EMB_IN)
    xc = xc.transpose(0, 2, 1, 3).reshape(2 * NC2, EMB_IN)  # (chain,t,b),f
    xTm[:320] = xc.T
    xTm[320] = 1.0
    wihm = np.empty((KD, 1024), np.float32)
    wihm[:320] = _perm_gates(Wih_d).T
    wihm[320] = _perm_gates(b_d.reshape(-1, 1))[:, 0]
    whhm = _perm_gates(Whh_d).T                      # (256, 1024)
    whh8 = np.ascontiguousarray(
        whhm.reshape(2, 128, 1024).transpose(1, 0, 2).reshape(128, 2048)
    ).astype(f8np)
    return {"xT": xTm.astype(f8np), "wih": wihm.astype(f8np), "whhT": whh8,
            "wtagT": np.ascontiguousarray(Wtag_rows).astype(f8np)}


def kernel(char_tensor, token_tensor, tags, mask, emb,
           cWih_f, cWhh_f, cb_f, cWih_b, cWhh_b, cb_b,
           wWih_f, wWhh_f, wb_f, wWih_b, wWhh_b, wb_b,
           Wtag, btag, start_t, end_t, trans):
    f32 = lambda a: np.asarray(a, np.float32)
    char_tensor = f32(char_tensor)
    emb = f32(emb)
    token_tensor = np.asarray(token_tensor).astype(np.int64)
    tags_i = np.asarray(tags).astype(np.int64)
    mask_b = np.asarray(mask).astype(bool)

    # --- char BiLSTM (tiny) + embedding gather on host ---
    cf = _lstm_dir_host(char_tensor, f32(cWih_f), f32(cWhh_f), f32(cb_f))
    cb = _lstm_dir_host(char_tensor[:, ::-1], f32(cWih_b), f32(cWhh_b),
                        f32(cb_b))[:, ::-1]
    word_emb = emb[token_tensor]                                  # (B,T,300)
    x = np.concatenate([cf, cb, word_emb], axis=2)                # (B,T,320)

    WtagT = np.ascontiguousarray(f32(Wtag).T)                     # (512, 20)
    per_core = []
    for g in range(4):
        xg_ = x[g * BL2:(g + 1) * BL2]
        per_core.append(_pack_dir(xg_, f32(wWih_f), f32(wb_f), f32(wWhh_f),
                                  WtagT[0:256], flip=False))
        per_core.append(_pack_dir(xg_, f32(wWih_b), f32(wb_b), f32(wWhh_b),
                                  WtagT[256:512], flip=True))
    # assemble the sharded slabs once here (host prep) so the device call
    # does no per-call concatenation
    in_maps = {n: np.concatenate([m[n] for m in per_core], axis=0)
               for n in per_core[0]}

    if "nc" not in _CACHE:
        _CACHE["nc"] = _build_nc()
    _CACHE["last_in_maps"] = in_maps
    # First exec on a freshly-compiled NEFF occasionally hits a transient
    # failure on this axon tunnel; retry (fresh build on second failure).
    res = None
    for attempt in range(3):
        try:
            res = _run_device(in_maps)
            break
        except Exception:
            if attempt == 2:
                raise
            import time as _time
            _time.sleep(5)
            _CACHE.pop("runner", None)
            if attempt == 1:
                _CACHE.pop("nc", None)
                _CACHE["nc"] = _build_nc()

    em = np.empty((B, T, K), np.float32)
    for g in range(4):
        ef = np.asarray(res[2 * g]["em"], np.float32)
        eb = np.asarray(res[2 * g + 1]["em"], np.float32)
        # col = chain*2048 + t*8 + b -> (chain,t,b) -> (16 seqs, T)
        ef = ef.T.reshape(2, T, CW, K).transpose(0, 2, 1, 3).reshape(
            BL2, T, K)
        eb = eb.T.reshape(2, T, CW, K).transpose(0, 2, 1, 3).reshape(
            BL2, T, K)[:, ::-1]                     # un-flip time
        em[g * BL2:(g + 1) * BL2] = ef + eb
    em += f32(btag)

    # --- CRF NLL on host (generic mask support) ---
    em = np.swapaxes(em, 0, 1)                                    # (T,B,K)
    tg = np.swapaxes(tags_i, 0, 1)
    m = np.swapaxes(mask_b, 0, 1).astype(np.float32)
    start_t, end_t, trans = f32(start_t), f32(end_t), f32(trans)
    bidx = np.arange(B)
    e_sc = np.take_along_axis(em, tg[..., None], axis=-1)[..., 0]  # (T,B)
    num = start_t[tg[0]] + e_sc[0]
    num = num + np.sum((trans[tg[:-1], tg[1:]] + e_sc[1:]) * m[1:], axis=0)
    last = (np.sum(m, axis=0) - 1).astype(np.int64)
    num = num + end_t[tg[last, bidx]]
    alpha = start_t[None, :] + em[0]
    for t in range(1, T):
        nxt = _logsumexp(alpha[:, :, None] + trans[None, :, :]
                         + em[t][:, None, :], axis=1)
        alpha = np.where(m[t][:, None] > 0, nxt, alpha)
    den = _logsumexp(alpha + end_t[None, :], axis=1)
    return np.float32(-np.sum(num - den))


# revision 13
# speedup vs baseline: 411.1078x; 1.1472x over previous
"""BiLSTM-CRF loss for nn_BiLSTM_CRF_68152541053203 on 8 TRN2 NeuronCores.

Sharding: batch x direction. B=64 splits into 4 groups of 16 sequences; each
group gets a core pair: core 2g runs the forward word-LSTM direction, core
2g+1 the backward direction (on host-time-flipped inputs, so the SPMD kernel
is identical). Each core computes on-device, SBUF-resident:
    xg = x @ Wih_d.T + b_d          (projected in 8-step chunks into PSUM)
    single-direction LSTM recurrence, 256 steps, fp8 DoubleRow matmuls
    em_part = seq_d @ Wtag_d.T      -> (20, 4096) fp8
Host: char BiLSTM + embedding gather (tiny), sums the two partial emissions
per group (+btag, bwd part time-unflipped), and runs the CRF forward scan
(generic mask support).

Key performance structure (the per-step LSTM recurrence is dependency-
latency-bound on TRN2: every dependency edge costs ~0.5us of semaphore
round-trip, so the design minimizes serially dependent instructions per
step - 6 edges: matmul -> sigmoid -> product -> c-update -> tanh -> h-mul):
  * The input projection xg = x@Wih.T runs on the (otherwise idle) PE in
    8-step chunks, directly into PSUM (start=True); the per-step Whh
    recurrence matmul then accumulates on top (start=False - PSUM has
    per-element has_written bits), so no separate gate-sum add is needed
    and the Activation engine reads gate pre-activations straight from
    PSUM. Chunks are double-buffered (2 two-bank PSUM tiles, parity).
  * tanh(x) = 2*sigmoid(2x) - 1: the factor 2 is folded into the g-gate
    rows of Wih/Whh/bias on the host, so ONE sigmoid activation covers all
    four gates.
  * The cell state is kept pre-halved (ct = c/2), which turns the 2s-1
    fixup + c-update into two independent DVE products (each depending
    only on the sigmoid output) plus one add; tanh(c) is recovered free
    via the activation's input pre-scale (tanh(2*ct)).

Device layouts (per core, 16 seqs, T=256):
  xT    (321, 4096) fp8e4m3: col = t*16 + b (time-major); rows 0:320
        features, row 320 ones (bias row trick); bwd cores receive x
        time-reversed
  wih   (321, 1024) fp8e4m3: cols = this direction's gates, order [i,f,o,g],
        g-gate cols pre-scaled by 2
  whhT  (128, 2048) fp8e4m3: [p, k*1024+g] = WhhT_d[k*128+p, g] (DoubleRow),
        g-gate cols pre-scaled by 2
  wtagT (256, 20)  fp8e4m3: this direction's 256 rows of Wtag.T
  em    (20, 4096) fp8e4m3 output (partial emissions, no btag)
On-chip: sq (128, 2*4096) fp8 (col = k*4096 + t*16 + b), which doubles as
the next step's matmul rhs and the emission matmul rhs; gate/cell state
f32 tiles.
"""

import numpy as np

import concourse.bacc as bacc
import concourse.mybir as mybir
import concourse.tile as tile

N_CORES = 8
B, T = 64, 256
CIN, CH = 25, 10
EMB_IN, H = 320, 256
K = 20
BL2 = 16                      # sequences per core (4 groups x 2 dirs)
CW = 8                        # sequences per chain (2 chains per core)
NC2 = CW * T                  # 2048 cols per chain
NT = BL2 * T                  # 4096
KD = EMB_IN + 1
S = 8                         # steps per PSUM projection chunk
NG = T // S                   # 32 chunks
AF = mybir.ActivationFunctionType

_CACHE = {}


def _build_nc(repeat=1):
    from concourse.alu_op_type import AluOpType as ALU
    bf = mybir.dt.bfloat16
    f8 = mybir.dt.float8e4
    f32 = mybir.dt.float32
    nc = bacc.Bacc("TRN2", target_bir_lowering=False, debug=False,
                   num_devices=N_CORES)
    xT = nc.dram_tensor("xT", [KD, NT], f8, kind="ExternalInput").ap()
    wih = nc.dram_tensor("wih", [KD, 1024], f8, kind="ExternalInput").ap()
    whhT = nc.dram_tensor("whhT", [128, 2048], f8, kind="ExternalInput").ap()
    wtagT = nc.dram_tensor("wtagT", [H, K], f8, kind="ExternalInput").ap()
    em = nc.dram_tensor("em", [K, NT], f8, kind="ExternalOutput").ap()

    with tile.TileContext(nc) as tc:
        for _rep in range(repeat):
            _emit_body(nc, tc, xT, wih, whhT, wtagT, em, f8, f32, ALU)
    nc.compile()
    return nc


def _emit_body(nc, tc, xT, wih, whhT, wtagT, em, f8, f32, ALU):
    DR = mybir.MatmulPerfMode.DoubleRow
    with (
        tc.tile_pool(name="per", bufs=1) as per,
        tc.tile_pool(name="st", bufs=2) as st,
    ):
        wih01 = per.tile([128, 2 * 1024], f8, tag="wih01", name="wih01")
        nc.gpsimd.dma_start(wih01[:, 0:1024], wih[0:128, :])
        nc.gpsimd.dma_start(wih01[:, 1024:2048], wih[128:256, :])
        wih01r = wih01[:].rearrange("p (k g) -> p k g", k=2)
        wih2 = per.tile([65, 1024], f8, tag="wih2", name="wih2")
        nc.gpsimd.dma_start(wih2[:], wih[256:321, :])
        xT01 = per.tile([128, 2 * NT], f8, tag="xT01", name="xT01")
        nc.gpsimd.dma_start(xT01[:, 0:NT], xT[0:128, :])
        nc.gpsimd.dma_start(xT01[:, NT:2 * NT], xT[128:256, :])
        xT01r = xT01[:].rearrange("p (k n) -> p k n", k=2)
        xT2 = per.tile([65, NT], f8, tag="xT2", name="xT2")
        nc.gpsimd.dma_start(xT2[:], xT[256:321, :])
        whh8 = per.tile([128, 2048], f8, tag="whh8", name="whh8")
        nc.gpsimd.dma_start(whh8[:], whhT[:, :])
        whh8r = whh8[:].rearrange("p (k g) -> p k g", k=2)
        wt8 = per.tile([128, 2 * K], f8, tag="wt8", name="wt8")
        nc.gpsimd.dma_start(wt8[:, 0:K], wtagT[0:128, :])
        nc.gpsimd.dma_start(wt8[:, K:2 * K], wtagT[128:256, :])
        wt8r = wt8[:].rearrange("p (k n) -> p k n", k=2)

        # h history; col = k*4096 + t*16 + b
        sq = per.tile([128, 2 * NT], f8, tag="sq", name="sq")
        sq4 = sq[:].rearrange("p (k n) -> p k n", k=2)
        # gate tile: [i 0:32 | f 32:64 | o 64:96 | g 96:128]
        ga2 = per.tile([128, 128], f32, tag="ga2", name="ga2")
        # [scratch 0:32 | ct 32:64]
        gc2 = per.tile([128, 64], f32, tag="gc2", name="gc2")
        th2 = per.tile([128, 32], f32, tag="th2", name="th2")
        nc.vector.memset(gc2[:, 32:64], 0.0)

        with tc.tile_pool(name="prm", bufs=1, space="PSUM") as prm:
            # 2 two-bank tiles (group parity); each holds an 8-step chunk of
            # gate pre-activations [c*128 + t_rel*16 + b]. Chunk regions are
            # 128 f32 so none crosses a bank boundary. Separate parity tiles
            # so tile-granular dependency tracking doesn't serialize one
            # chunk's activations behind the next chunk's projections.
            psq = [prm.tile([128, 1024], f32, tag=f"ps{q}", name=f"ps{q}")
                   for q in range(2)]
            psvq = [p[:] for p in psq]
            psrq = [p[:].rearrange("p (c n) -> p c n", c=8) for p in psq]

            def proj(g, c):
                """Project xg for chunk g (8 steps), gate-chunk c, into
                PSUM parity tile g%2."""
                dst = psvq[g % 2][:, c * 128:(c + 1) * 128]
                cols = slice(g * 128, (g + 1) * 128)
                nc.tensor.matmul(dst, wih01r[:, :, c * 128:(c + 1) * 128],
                                 xT01r[:, :, cols], perf_mode=DR,
                                 start=True, stop=False)
                nc.tensor.matmul(dst, wih2[:, c * 128:(c + 1) * 128],
                                 xT2[:, cols], start=False, stop=True)

            for c in range(8):
                proj(0, c)

            for g in range(NG):
                for tr in range(S):
                    t = g * S + tr
                    q = g % 2
                    # --- PE: recurrence matmuls accumulate onto xg ---
                    if t > 0:
                        for c in range(8):
                            nc.tensor.matmul(
                                psvq[q][:, c * 128 + tr * 16:
                                        c * 128 + tr * 16 + 16],
                                whh8r[:, :, c * 128:(c + 1) * 128],
                                sq4[:, :, (t - 1) * 16:t * 16],
                                perf_mode=DR, start=False, stop=True,
                                skip_group_check=True)
                    # --- PE: next chunk's projection, spread over steps ---
                    if g + 1 < NG:
                        proj(g + 1, tr)
                    # --- Act: one sigmoid over all gates (2x folded in g) ---
                    nc.scalar.activation(
                        ga2[:],
                        psrq[q][:, :, tr * 16:(tr + 1) * 16], AF.Sigmoid)
                    # cell state is kept pre-halved (ct = c/2):
                    #   ct' = f*ct + (sg - 1/2)*i   [= c'/2]
                    #   tanh(c') = tanh(2*ct')      [free via Act pre-scale]
                    # so the two products depend only on the sigmoid output
                    # (no serial ghat->mul link) and the 2s-1 fixup vanishes.
                    cc = gc2[:, 32:64]
                    t12 = st.tile([128, 64], f32, tag="t12", name="t12")
                    nc.vector.tensor_mul(t12[:, 32:64], ga2[:, 32:64], cc)
                    nc.vector.scalar_tensor_tensor(
                        t12[:, 0:32], ga2[:, 96:128], -0.5,
                        ga2[:, 0:32], ALU.add, ALU.mult)
                    nc.vector.tensor_add(cc, t12[:, 0:32], t12[:, 32:64])
                    nc.scalar.activation(th2[:], cc, AF.Tanh, scale=2.0)
                    nc.vector.tensor_mul(
                        sq4[:, :, t * 16:(t + 1) * 16],
                        ga2[:, 64:96], th2[:])

        # partial emissions: em = seq_d @ Wtag_d.T (DoubleRow over k)
        em_sb = per.tile([K, NT], f8, tag="em", name="em")
        with tc.tile_pool(name="pe", bufs=2, space="PSUM") as pe:
            for nb in range(8):
                ps = pe.tile([K, 512], f32, tag="eps", name="eps")
                for k in range(2):
                    nc.tensor.matmul(
                        ps[:], wt8r[:, k, :],
                        sq4[:, k, nb * 512:(nb + 1) * 512],
                        start=(k == 0), stop=(k == 1))
                dst = em_sb[:, nb * 512:(nb + 1) * 512]
                if nb % 2 == 0:
                    nc.vector.tensor_copy(dst, ps[:])
                else:
                    nc.scalar.copy(dst, ps[:])
            nc.gpsimd.dma_start(em[:, :], em_sb[:])


def _make_runner(nc):
    """Build the jitted PJRT executor once (run_bass_via_pjrt re-traces on
    every call; caching the jit + shard_map saves that per-call cost)."""
    import jax
    from jax.sharding import Mesh, PartitionSpec
    from jax.experimental.shard_map import shard_map
    from concourse import bass2jax
    bass2jax.install_neuronx_cc_hook()
    assert nc.dbg_addr is None
    pid_name = (nc.partition_id_tensor.name
                if nc.partition_id_tensor else None)

    in_names, out_names, out_avals, zero_outs = [], [], [], []
    for alloc in nc.m.functions[0].allocations:
        if not isinstance(alloc, mybir.MemoryLocationSet):
            continue
        name = alloc.memorylocations[0].name
        if alloc.kind == "ExternalInput":
            if name != pid_name:
                in_names.append(name)
        elif alloc.kind == "ExternalOutput":
            out_names.append(name)
            shape = tuple(alloc.tensor_shape)
            dtype = mybir.dt.np(alloc.dtype)
            out_avals.append(jax.core.ShapedArray(shape, dtype))
            zero_outs.append(np.zeros(shape, dtype))
    n_params = len(in_names)
    all_names = in_names + out_names
    if pid_name is not None:
        all_names = all_names + [pid_name]

    def _body(*args):
        operands = list(args)
        if pid_name is not None:
            operands.append(bass2jax.partition_id_tensor())
        outs = bass2jax._bass_exec_p.bind(
            *operands, out_avals=tuple(out_avals), in_names=tuple(all_names),
            out_names=tuple(out_names), lowering_input_output_aliases=(),
            sim_require_finite=True, sim_require_nnan=True, nc=nc)
        return tuple(outs)

    devices = jax.devices()[:N_CORES]
    mesh = Mesh(np.asarray(devices), ("core",))
    specs = (PartitionSpec("core"),) * (n_params + len(out_names))
    # No donation: the kernel writes every element of its outputs, so the
    # zero buffers can be passed persistently (lets the bench path reuse
    # device-resident buffers across calls).
    sharded = jax.jit(
        shard_map(_body, mesh=mesh, in_specs=specs,
                  out_specs=(PartitionSpec("core"),) * len(out_names),
                  check_rep=False),
        keep_unused=True)

    out_sharding = jax.sharding.NamedSharding(mesh, PartitionSpec("core"))

    def put(in_maps):
        """device_put the input slabs + persistent zero output buffers."""
        if not isinstance(in_maps, dict):
            in_maps = {n: np.concatenate([m[n] for m in in_maps], axis=0)
                       for n in in_maps[0]}
        dev_in = [jax.device_put(in_maps[n], out_sharding) for n in in_names]
        dev_zero = [jax.device_put(
            np.zeros((N_CORES * z.shape[0], *z.shape[1:]), z.dtype),
            out_sharding) for z in zero_outs]
        args = dev_in + dev_zero
        jax.block_until_ready(args)
        return args

    def exec_async(args):
        return sharded(*args)

    def run(in_maps):
        outs = exec_async(put(in_maps))
        return [{n: np.asarray(outs[i]).reshape(
                    N_CORES, *out_avals[i].shape)[c]
                 for i, n in enumerate(out_names)}
                for c in range(N_CORES)]

    run.put = put
    run.exec_async = exec_async
    return run


def _run_device(in_maps):
    if "runner" not in _CACHE:
        _CACHE["runner"] = _make_runner(_CACHE["nc"])
    return _CACHE["runner"](in_maps)


def _perm_gates(W):
    """PyTorch gate-row order [i,f,g,o] -> kernel order [i,f,o,g], with the
    g-gate rows scaled by 2 (tanh(x) = 2*sigmoid(2x) - 1 folding).

    W: (1024, ...) single-direction gate-stacked array."""
    return np.concatenate([W[0:512], W[768:1024], 2.0 * W[512:768]], axis=0)


def _sigmoid(x):
    return 1.0 / (1.0 + np.exp(-x))


def _lstm_dir_host(x, Wih, Whh, b):
    """Small (char) LSTM on host. x: (B,T,I) -> (B,T,Hd)."""
    xg = np.einsum('bti,gi->btg', x, Wih, optimize=True) + b
    xg = xg.astype(np.float32)
    Bs, Ts, G = xg.shape
    Hd = G // 4
    WhhT = np.ascontiguousarray(Whh.T)
    h = np.zeros((Bs, Hd), np.float32)
    c = np.zeros((Bs, Hd), np.float32)
    out = np.empty((Bs, Ts, Hd), np.float32)
    for t in range(Ts):
        g = xg[:, t] + h @ WhhT
        i = _sigmoid(g[:, :Hd])
        f = _sigmoid(g[:, Hd:2 * Hd])
        gg = np.tanh(g[:, 2 * Hd:3 * Hd])
        o = _sigmoid(g[:, 3 * Hd:])
        c = f * c + i * gg
        h = o * np.tanh(c)
        out[:, t] = h
    return out


def _logsumexp(a, axis):
    m = np.max(a, axis=axis, keepdims=True)
    return (m + np.log(np.sum(np.exp(a - m), axis=axis,
                              keepdims=True))).squeeze(axis)


def _pack_dir(x_grp, Wih_d, b_d, Whh_d, Wtag_rows, flip):
    """Build one core's in_map. x_grp: (16,T,320) f32."""
    import ml_dtypes
    f8np = ml_dtypes.float8_e4m3
    xs = x_grp[:, ::-1] if flip else x_grp
    xTm = np.empty((KD, NT), np.float32)
    # col = t*16 + b: (16 seqs, T, 320) -> time-major
    xc = np.ascontiguousarray(xs).transpose(1, 0, 2).reshape(NT, EMB_IN)
    xTm[:320] = xc.T
    xTm[320] = 1.0
    wihm = np.empty((KD, 1024), np.float32)
    wihm[:320] = _perm_gates(Wih_d).T
    wihm[320] = _perm_gates(b_d.reshape(-1, 1))[:, 0]
    whhm = _perm_gates(Whh_d).T                      # (256, 1024)
    whh8 = np.ascontiguousarray(
        whhm.reshape(2, 128, 1024).transpose(1, 0, 2).reshape(128, 2048)
    ).astype(f8np)
    return {"xT": xTm.astype(f8np), "wih": wihm.astype(f8np), "whhT": whh8,
            "wtagT": np.ascontiguousarray(Wtag_rows).astype(f8np)}


def kernel(char_tensor, token_tensor, tags, mask, emb,
           cWih_f, cWhh_f, cb_f, cWih_b, cWhh_b, cb_b,
           wWih_f, wWhh_f, wb_f, wWih_b, wWhh_b, wb_b,
           Wtag, btag, start_t, end_t, trans):
    f32 = lambda a: np.asarray(a, np.float32)
    char_tensor = f32(char_tensor)
    emb = f32(emb)
    token_tensor = np.asarray(token_tensor).astype(np.int64)
    tags_i = np.asarray(tags).astype(np.int64)
    mask_b = np.asarray(mask).astype(bool)

    # --- char BiLSTM (tiny) + embedding gather on host ---
    cf = _lstm_dir_host(char_tensor, f32(cWih_f), f32(cWhh_f), f32(cb_f))
    cb = _lstm_dir_host(char_tensor[:, ::-1], f32(cWih_b), f32(cWhh_b),
                        f32(cb_b))[:, ::-1]
    word_emb = emb[token_tensor]                                  # (B,T,300)
    x = np.concatenate([cf, cb, word_emb], axis=2)                # (B,T,320)

    WtagT = np.ascontiguousarray(f32(Wtag).T)                     # (512, 20)
    per_core = []
    for g in range(4):
        xg_ = x[g * BL2:(g + 1) * BL2]
        per_core.append(_pack_dir(xg_, f32(wWih_f), f32(wb_f), f32(wWhh_f),
                                  WtagT[0:256], flip=False))
        per_core.append(_pack_dir(xg_, f32(wWih_b), f32(wb_b), f32(wWhh_b),
                                  WtagT[256:512], flip=True))
    # assemble the sharded slabs once here (host prep) so the device call
    # does no per-call concatenation
    in_maps = {n: np.concatenate([m[n] for m in per_core], axis=0)
               for n in per_core[0]}

    if "nc" not in _CACHE:
        _CACHE["nc"] = _build_nc()
    _CACHE["last_in_maps"] = in_maps
    # First exec on a freshly-compiled NEFF occasionally hits a transient
    # failure on this axon tunnel; retry (fresh build on second failure).
    res = None
    for attempt in range(3):
        try:
            res = _run_device(in_maps)
            break
        except Exception:
            if attempt == 2:
                raise
            import time as _time
            _time.sleep(5)
            _CACHE.pop("runner", None)
            if attempt == 1:
                _CACHE.pop("nc", None)
                _CACHE["nc"] = _build_nc()

    em = np.empty((B, T, K), np.float32)
    for g in range(4):
        ef = np.asarray(res[2 * g]["em"], np.float32)
        eb = np.asarray(res[2 * g + 1]["em"], np.float32)
        # col = t*16 + b -> (t,b) -> (16 seqs, T)
        ef = ef.T.reshape(T, BL2, K).transpose(1, 0, 2)
        eb = eb.T.reshape(T, BL2, K).transpose(1, 0, 2)[:, ::-1]  # un-flip
        em[g * BL2:(g + 1) * BL2] = ef + eb
    em += f32(btag)

    # --- CRF NLL on host (generic mask support) ---
    em = np.swapaxes(em, 0, 1)                                    # (T,B,K)
    tg = np.swapaxes(tags_i, 0, 1)
    m = np.swapaxes(mask_b, 0, 1).astype(np.float32)
    start_t, end_t, trans = f32(start_t), f32(end_t), f32(trans)
    bidx = np.arange(B)
    e_sc = np.take_along_axis(em, tg[..., None], axis=-1)[..., 0]  # (T,B)
    num = start_t[tg[0]] + e_sc[0]
    num = num + np.sum((trans[tg[:-1], tg[1:]] + e_sc[1:]) * m[1:], axis=0)
    last = (np.sum(m, axis=0) - 1).astype(np.int64)
    num = num + end_t[tg[last, bidx]]
    alpha = start_t[None, :] + em[0]
    for t in range(1, T):
        nxt = _logsumexp(alpha[:, :, None] + trans[None, :, :]
                         + em[t][:, None, :], axis=1)
        alpha = np.where(m[t][:, None] > 0, nxt, alpha)
    den = _logsumexp(alpha + end_t[None, :], axis=1)
    return np.float32(-np.sum(num - den))


# revision 18
# speedup vs baseline: 423.1497x; 1.0293x over previous
"""BiLSTM-CRF loss for nn_BiLSTM_CRF_68152541053203 on 8 TRN2 NeuronCores.

Sharding: batch x direction. B=64 splits into 4 groups of 16 sequences; each
group gets a core pair: core 2g runs the forward word-LSTM direction, core
2g+1 the backward direction (on host-time-flipped inputs, so the SPMD kernel
is identical). Each core computes on-device, SBUF-resident:
    xg = x @ Wih_d.T + b_d          (projected in 8-step chunks into PSUM)
    single-direction LSTM recurrence, 256 steps, fp8 DoubleRow matmuls
    em_part = seq_d @ Wtag_d.T      -> (20, 4096) fp8
Host: char BiLSTM + embedding gather (tiny), sums the two partial emissions
per group (+btag, bwd part time-unflipped), and runs the CRF forward scan
(generic mask support).

Key performance structure (the per-step LSTM recurrence is dependency-
latency-bound on TRN2: every dependency edge costs ~0.5us of semaphore
round-trip, so the design minimizes serially dependent instructions per
step - 6 edges: matmul -> sigmoid -> product -> c-update -> tanh -> h-mul):
  * The input projection xg = x@Wih.T runs on the (otherwise idle) PE in
    8-step chunks, directly into PSUM (start=True); the per-step Whh
    recurrence matmul then accumulates on top (start=False - PSUM has
    per-element has_written bits), so no separate gate-sum add is needed
    and the Activation engine reads gate pre-activations straight from
    PSUM. Chunks are double-buffered (2 two-bank PSUM tiles, parity).
  * tanh(x) = 2*sigmoid(2x) - 1: the factor 2 is folded into the g-gate
    rows of Wih/Whh/bias on the host, so ONE sigmoid activation covers all
    four gates.
  * The cell state is kept pre-halved (ct = c/2), which turns the 2s-1
    fixup + c-update into two independent DVE products (each depending
    only on the sigmoid output) plus one add; tanh(c) is recovered free
    via the activation's input pre-scale (tanh(2*ct)).

Device layouts (per core, 16 seqs, T=256):
  xT    (321, 4096) fp8e4m3: col = t*16 + b (time-major); rows 0:320
        features, row 320 ones (bias row trick); bwd cores receive x
        time-reversed
  wih   (321, 1024) fp8e4m3: cols = this direction's gates, order [i,f,o,g],
        g-gate cols pre-scaled by 2
  whhT  (128, 2048) fp8e4m3: [p, k*1024+g] = WhhT_d[k*128+p, g] (DoubleRow),
        g-gate cols pre-scaled by 2
  wtagT (256, 20)  fp8e4m3: this direction's 256 rows of Wtag.T
  em    (20, 4096) fp8e4m3 output (partial emissions, no btag)
On-chip: sq (128, 2*4096) fp8 (col = k*4096 + t*16 + b), which doubles as
the next step's matmul rhs and the emission matmul rhs; gate/cell state
f32 tiles.
"""

import numpy as np

import concourse.bacc as bacc
import concourse.mybir as mybir
import concourse.tile as tile

N_CORES = 8
B, T = 64, 256
CIN, CH = 25, 10
EMB_IN, H = 320, 256
K = 20
BL2 = 16                      # sequences per core (4 groups x 2 dirs)
CW = 8                        # sequences per chain (2 chains per core)
NC2 = CW * T                  # 2048 cols per chain
NT = BL2 * T                  # 4096
KD = EMB_IN + 1
S = 8                         # steps per PSUM projection chunk
NG = T // S                   # 32 chunks
AF = mybir.ActivationFunctionType

_CACHE = {}


def _build_nc(repeat=1):
    from concourse.alu_op_type import AluOpType as ALU
    bf = mybir.dt.bfloat16
    f8 = mybir.dt.float8e4
    f32 = mybir.dt.float32
    nc = bacc.Bacc("TRN2", target_bir_lowering=False, debug=False,
                   num_devices=N_CORES)
    xT = nc.dram_tensor("xT", [KD, NT], f8, kind="ExternalInput").ap()
    wih = nc.dram_tensor("wih", [KD, 1024], f8, kind="ExternalInput").ap()
    whhT = nc.dram_tensor("whhT", [128, 2048], f8, kind="ExternalInput").ap()
    wtagT = nc.dram_tensor("wtagT", [H, K], f8, kind="ExternalInput").ap()
    em = nc.dram_tensor("em", [K, NT], f8, kind="ExternalOutput").ap()

    with tile.TileContext(nc) as tc:
        for _rep in range(repeat):
            _emit_body(nc, tc, xT, wih, whhT, wtagT, em, f8, f32, ALU)
    nc.compile()
    return nc


def _emit_body(nc, tc, xT, wih, whhT, wtagT, em, f8, f32, ALU):
    DR = mybir.MatmulPerfMode.DoubleRow
    with (
        tc.tile_pool(name="per", bufs=1) as per,
        tc.tile_pool(name="st", bufs=2) as st,
    ):
        wih01 = per.tile([128, 2 * 1024], f8, tag="wih01", name="wih01")
        nc.gpsimd.dma_start(wih01[:, 0:1024], wih[0:128, :])
        nc.gpsimd.dma_start(wih01[:, 1024:2048], wih[128:256, :])
        wih01r = wih01[:].rearrange("p (k g) -> p k g", k=2)
        wih2 = per.tile([65, 1024], f8, tag="wih2", name="wih2")
        nc.gpsimd.dma_start(wih2[:], wih[256:321, :])
        xT01 = per.tile([128, 2 * NT], f8, tag="xT01", name="xT01")
        nc.gpsimd.dma_start(xT01[:, 0:NT], xT[0:128, :])
        nc.gpsimd.dma_start(xT01[:, NT:2 * NT], xT[128:256, :])
        xT01r = xT01[:].rearrange("p (k n) -> p k n", k=2)
        xT2 = per.tile([65, NT], f8, tag="xT2", name="xT2")
        nc.gpsimd.dma_start(xT2[:], xT[256:321, :])
        whh8 = per.tile([128, 2048], f8, tag="whh8", name="whh8")
        nc.gpsimd.dma_start(whh8[:], whhT[:, :])
        whh8r = whh8[:].rearrange("p (k g) -> p k g", k=2)
        wt8 = per.tile([128, 2 * K], f8, tag="wt8", name="wt8")
        nc.gpsimd.dma_start(wt8[:, 0:K], wtagT[0:128, :])
        nc.gpsimd.dma_start(wt8[:, K:2 * K], wtagT[128:256, :])
        wt8r = wt8[:].rearrange("p (k n) -> p k n", k=2)

        # h history; col = k*4096 + t*16 + b
        sq = per.tile([128, 2 * NT], f8, tag="sq", name="sq")
        sq4 = sq[:].rearrange("p (k n) -> p k n", k=2)
        # gates + cell state in ONE tile so (g|cb) and (i|f) are contiguous
        # operand pairs: [i 0:32 | f 32:64 | o 64:96 | g 96:128 | cb 128:160]
        # where cb = c/2 + 1/2.
        gac = per.tile([128, 160], f32, tag="gac", name="gac")
        th2 = per.tile([128, 32], f32, tag="th2", name="th2")
        nm1 = per.tile([128, 1], f32, tag="nm1", name="nm1")
        nc.vector.memset(gac[:, 128:160], 0.5)
        nc.vector.memset(nm1[:], -1.0)

        with tc.tile_pool(name="prm", bufs=1, space="PSUM") as prm:
            # 2 two-bank tiles (group parity); each holds an 8-step chunk of
            # gate pre-activations [c*128 + t_rel*16 + b]. Chunk regions are
            # 128 f32 so none crosses a bank boundary. Separate parity tiles
            # so tile-granular dependency tracking doesn't serialize one
            # chunk's activations behind the next chunk's projections.
            psq = [prm.tile([128, 1024], f32, tag=f"ps{q}", name=f"ps{q}")
                   for q in range(2)]
            psvq = [p[:] for p in psq]
            psrq = [p[:].rearrange("p (c n) -> p c n", c=8) for p in psq]

            def proj(g, c):
                """Project xg for chunk g (8 steps), gate-chunk c, into
                PSUM parity tile g%2."""
                dst = psvq[g % 2][:, c * 128:(c + 1) * 128]
                cols = slice(g * 128, (g + 1) * 128)
                nc.tensor.matmul(dst, wih01r[:, :, c * 128:(c + 1) * 128],
                                 xT01r[:, :, cols], perf_mode=DR,
                                 start=True, stop=False)
                nc.tensor.matmul(dst, wih2[:, c * 128:(c + 1) * 128],
                                 xT2[:, cols], start=False, stop=True)

            for c in range(8):
                proj(0, c)

            for g in range(NG):
                for tr in range(S):
                    t = g * S + tr
                    q = g % 2
                    # --- PE: recurrence matmuls accumulate onto xg ---
                    if t > 0:
                        for c in range(8):
                            nc.tensor.matmul(
                                psvq[q][:, c * 128 + tr * 16:
                                        c * 128 + tr * 16 + 16],
                                whh8r[:, :, c * 128:(c + 1) * 128],
                                sq4[:, :, (t - 1) * 16:t * 16],
                                perf_mode=DR, start=False, stop=True,
                                skip_group_check=True)
                    # --- PE: next chunk's projection, spread over steps ---
                    if g + 1 < NG:
                        proj(g + 1, tr)
                    # --- Act: one sigmoid over all gates (2x folded in g) ---
                    nc.scalar.activation(
                        gac[:, 0:128],
                        psrq[q][:, :, tr * 16:(tr + 1) * 16], AF.Sigmoid)
                    # Cell state kept as cb = c/2 + 1/2, so ONE fused DVE op
                    # computes both cell-update products:
                    #   ((g|cb) - 1/2) * (i|f) = ((sg-1/2)*i | f*(c/2))
                    # (sg - 1/2 = tanh(g-arg)/2 since the g-gate weights
                    # carry the 2x fold) and a second fused op re-biases:
                    #   cb' = (u + 1/2) + f*(c/2)   [= c'/2 + 1/2]
                    # tanh(c') = tanh(2*cb' - 1) via the Act pre-scale/bias.
                    cb = gac[:, 128:160]
                    t12 = st.tile([128, 64], f32, tag="t12", name="t12")
                    nc.vector.scalar_tensor_tensor(
                        t12[:], gac[:, 96:160], -0.5,
                        gac[:, 0:64], ALU.add, ALU.mult)
                    nc.vector.scalar_tensor_tensor(
                        cb, t12[:, 0:32], 0.5,
                        t12[:, 32:64], ALU.add, ALU.add)
                    nc.scalar.activation(th2[:], cb, AF.Tanh,
                                         scale=2.0, bias=nm1[:])
                    nc.vector.tensor_mul(
                        sq4[:, :, t * 16:(t + 1) * 16],
                        gac[:, 64:96], th2[:])

        # partial emissions: em = seq_d @ Wtag_d.T (DoubleRow over k)
        em_sb = per.tile([K, NT], f8, tag="em", name="em")
        with tc.tile_pool(name="pe", bufs=2, space="PSUM") as pe:
            for nb in range(8):
                ps = pe.tile([K, 512], f32, tag="eps", name="eps")
                for k in range(2):
                    nc.tensor.matmul(
                        ps[:], wt8r[:, k, :],
                        sq4[:, k, nb * 512:(nb + 1) * 512],
                        start=(k == 0), stop=(k == 1))
                dst = em_sb[:, nb * 512:(nb + 1) * 512]
                if nb % 2 == 0:
                    nc.vector.tensor_copy(dst, ps[:])
                else:
                    nc.scalar.copy(dst, ps[:])
            nc.gpsimd.dma_start(em[:, :], em_sb[:])


def _make_runner(nc):
    """Build the jitted PJRT executor once (run_bass_via_pjrt re-traces on
    every call; caching the jit + shard_map saves that per-call cost)."""
    import jax
    from jax.sharding import Mesh, PartitionSpec
    from jax.experimental.shard_map import shard_map
    from concourse import bass2jax
    bass2jax.install_neuronx_cc_hook()
    assert nc.dbg_addr is None
    pid_name = (nc.partition_id_tensor.name
                if nc.partition_id_tensor else None)

    in_names, out_names, out_avals, zero_outs = [], [], [], []
    for alloc in nc.m.functions[0].allocations:
        if not isinstance(alloc, mybir.MemoryLocationSet):
            continue
        name = alloc.memorylocations[0].name
        if alloc.kind == "ExternalInput":
            if name != pid_name:
                in_names.append(name)
        elif alloc.kind == "ExternalOutput":
            out_names.append(name)
            shape = tuple(alloc.tensor_shape)
            dtype = mybir.dt.np(alloc.dtype)
            out_avals.append(jax.core.ShapedArray(shape, dtype))
            zero_outs.append(np.zeros(shape, dtype))
    n_params = len(in_names)
    all_names = in_names + out_names
    if pid_name is not None:
        all_names = all_names + [pid_name]

    def _body(*args):
        operands = list(args)
        if pid_name is not None:
            operands.append(bass2jax.partition_id_tensor())
        outs = bass2jax._bass_exec_p.bind(
            *operands, out_avals=tuple(out_avals), in_names=tuple(all_names),
            out_names=tuple(out_names), lowering_input_output_aliases=(),
            sim_require_finite=True, sim_require_nnan=True, nc=nc)
        return tuple(outs)

    devices = jax.devices()[:N_CORES]
    mesh = Mesh(np.asarray(devices), ("core",))
    specs = (PartitionSpec("core"),) * (n_params + len(out_names))
    # No donation: the kernel writes every element of its outputs, so the
    # zero buffers can be passed persistently (lets the bench path reuse
    # device-resident buffers across calls).
    sharded = jax.jit(
        shard_map(_body, mesh=mesh, in_specs=specs,
                  out_specs=(PartitionSpec("core"),) * len(out_names),
                  check_rep=False),
        keep_unused=True)

    out_sharding = jax.sharding.NamedSharding(mesh, PartitionSpec("core"))

    def put(in_maps):
        """device_put the input slabs + persistent zero output buffers."""
        if not isinstance(in_maps, dict):
            in_maps = {n: np.concatenate([m[n] for m in in_maps], axis=0)
                       for n in in_maps[0]}
        dev_in = [jax.device_put(in_maps[n], out_sharding) for n in in_names]
        dev_zero = [jax.device_put(
            np.zeros((N_CORES * z.shape[0], *z.shape[1:]), z.dtype),
            out_sharding) for z in zero_outs]
        args = dev_in + dev_zero
        jax.block_until_ready(args)
        return args

    def exec_async(args):
        return sharded(*args)

    def run(in_maps):
        outs = exec_async(put(in_maps))
        return [{n: np.asarray(outs[i]).reshape(
                    N_CORES, *out_avals[i].shape)[c]
                 for i, n in enumerate(out_names)}
                for c in range(N_CORES)]

    run.put = put
    run.exec_async = exec_async
    return run


def _run_device(in_maps):
    if "runner" not in _CACHE:
        _CACHE["runner"] = _make_runner(_CACHE["nc"])
    return _CACHE["runner"](in_maps)


def _perm_gates(W):
    """PyTorch gate-row order [i,f,g,o] -> kernel order [i,f,o,g], with the
    g-gate rows scaled by 2 (tanh(x) = 2*sigmoid(2x) - 1 folding).

    W: (1024, ...) single-direction gate-stacked array."""
    return np.concatenate([W[0:512], W[768:1024], 2.0 * W[512:768]], axis=0)


def _sigmoid(x):
    return 1.0 / (1.0 + np.exp(-x))


def _lstm_dir_host(x, Wih, Whh, b):
    """Small (char) LSTM on host. x: (B,T,I) -> (B,T,Hd)."""
    xg = np.einsum('bti,gi->btg', x, Wih, optimize=True) + b
    xg = xg.astype(np.float32)
    Bs, Ts, G = xg.shape
    Hd = G // 4
    WhhT = np.ascontiguousarray(Whh.T)
    h = np.zeros((Bs, Hd), np.float32)
    c = np.zeros((Bs, Hd), np.float32)
    out = np.empty((Bs, Ts, Hd), np.float32)
    for t in range(Ts):
        g = xg[:, t] + h @ WhhT
        i = _sigmoid(g[:, :Hd])
        f = _sigmoid(g[:, Hd:2 * Hd])
        gg = np.tanh(g[:, 2 * Hd:3 * Hd])
        o = _sigmoid(g[:, 3 * Hd:])
        c = f * c + i * gg
        h = o * np.tanh(c)
        out[:, t] = h
    return out


def _logsumexp(a, axis):
    m = np.max(a, axis=axis, keepdims=True)
    return (m + np.log(np.sum(np.exp(a - m), axis=axis,
                              keepdims=True))).squeeze(axis)


def _pack_dir(x_grp, Wih_d, b_d, Whh_d, Wtag_rows, flip):
    """Build one core's in_map. x_grp: (16,T,320) f32."""
    import ml_dtypes
    f8np = ml_dtypes.float8_e4m3
    xs = x_grp[:, ::-1] if flip else x_grp
    xTm = np.empty((KD, NT), np.float32)
    # col = t*16 + b: (16 seqs, T, 320) -> time-major
    xc = np.ascontiguousarray(xs).transpose(1, 0, 2).reshape(NT, EMB_IN)
    xTm[:320] = xc.T
    xTm[320] = 1.0
    wihm = np.empty((KD, 1024), np.float32)
    wihm[:320] = _perm_gates(Wih_d).T
    wihm[320] = _perm_gates(b_d.reshape(-1, 1))[:, 0]
    whhm = _perm_gates(Whh_d).T                      # (256, 1024)
    whh8 = np.ascontiguousarray(
        whhm.reshape(2, 128, 1024).transpose(1, 0, 2).reshape(128, 2048)
    ).astype(f8np)
    return {"xT": xTm.astype(f8np), "wih": wihm.astype(f8np), "whhT": whh8,
            "wtagT": np.ascontiguousarray(Wtag_rows).astype(f8np)}


def kernel(char_tensor, token_tensor, tags, mask, emb,
           cWih_f, cWhh_f, cb_f, cWih_b, cWhh_b, cb_b,
           wWih_f, wWhh_f, wb_f, wWih_b, wWhh_b, wb_b,
           Wtag, btag, start_t, end_t, trans):
    f32 = lambda a: np.asarray(a, np.float32)
    char_tensor = f32(char_tensor)
    emb = f32(emb)
    token_tensor = np.asarray(token_tensor).astype(np.int64)
    tags_i = np.asarray(tags).astype(np.int64)
    mask_b = np.asarray(mask).astype(bool)

    # --- char BiLSTM (tiny) + embedding gather on host ---
    cf = _lstm_dir_host(char_tensor, f32(cWih_f), f32(cWhh_f), f32(cb_f))
    cb = _lstm_dir_host(char_tensor[:, ::-1], f32(cWih_b), f32(cWhh_b),
                        f32(cb_b))[:, ::-1]
    word_emb = emb[token_tensor]                                  # (B,T,300)
    x = np.concatenate([cf, cb, word_emb], axis=2)                # (B,T,320)

    WtagT = np.ascontiguousarray(f32(Wtag).T)                     # (512, 20)
    per_core = []
    for g in range(4):
        xg_ = x[g * BL2:(g + 1) * BL2]
        per_core.append(_pack_dir(xg_, f32(wWih_f), f32(wb_f), f32(wWhh_f),
                                  WtagT[0:256], flip=False))
        per_core.append(_pack_dir(xg_, f32(wWih_b), f32(wb_b), f32(wWhh_b),
                                  WtagT[256:512], flip=True))
    # assemble the sharded slabs once here (host prep) so the device call
    # does no per-call concatenation
    in_maps = {n: np.concatenate([m[n] for m in per_core], axis=0)
               for n in per_core[0]}

    if "nc" not in _CACHE:
        _CACHE["nc"] = _build_nc()
    _CACHE["last_in_maps"] = in_maps
    # First exec on a freshly-compiled NEFF occasionally hits a transient
    # failure on this axon tunnel; retry (fresh build on second failure).
    res = None
    for attempt in range(3):
        try:
            res = _run_device(in_maps)
            break
        except Exception:
            if attempt == 2:
                raise
            import time as _time
            _time.sleep(5)
            _CACHE.pop("runner", None)
            if attempt == 1:
                _CACHE.pop("nc", None)
                _CACHE["nc"] = _build_nc()

    em = np.empty((B, T, K), np.float32)
    for g in range(4):
        ef = np.asarray(res[2 * g]["em"], np.float32)
        eb = np.asarray(res[2 * g + 1]["em"], np.float32)
        # col = t*16 + b -> (t,b) -> (16 seqs, T)
        ef = ef.T.reshape(T, BL2, K).transpose(1, 0, 2)
        eb = eb.T.reshape(T, BL2, K).transpose(1, 0, 2)[:, ::-1]  # un-flip
        em[g * BL2:(g + 1) * BL2] = ef + eb
    em += f32(btag)

    # --- CRF NLL on host (generic mask support) ---
    em = np.swapaxes(em, 0, 1)                                    # (T,B,K)
    tg = np.swapaxes(tags_i, 0, 1)
    m = np.swapaxes(mask_b, 0, 1).astype(np.float32)
    start_t, end_t, trans = f32(start_t), f32(end_t), f32(trans)
    bidx = np.arange(B)
    e_sc = np.take_along_axis(em, tg[..., None], axis=-1)[..., 0]  # (T,B)
    num = start_t[tg[0]] + e_sc[0]
    num = num + np.sum((trans[tg[:-1], tg[1:]] + e_sc[1:]) * m[1:], axis=0)
    last = (np.sum(m, axis=0) - 1).astype(np.int64)
    num = num + end_t[tg[last, bidx]]
    alpha = start_t[None, :] + em[0]
    for t in range(1, T):
        nxt = _logsumexp(alpha[:, :, None] + trans[None, :, :]
                         + em[t][:, None, :], axis=1)
        alpha = np.where(m[t][:, None] > 0, nxt, alpha)
    den = _logsumexp(alpha + end_t[None, :], axis=1)
    return np.float32(-np.sum(num - den))
